# revision 30
# baseline (speedup 1.0000x reference)
"""Trainium2 Bass kernel for GQA attention layer (Llama-style, prefill).

Full computation:  out = softmax((rope(x@wq) @ rope(x@wk)^T)*scale + causal) @ (x@wv) @ wo

Sharding: 8 cores = DP(2 batches) x TP(4 head-groups).  Core c = 4*b + g
handles batch b, q-heads [8g..8g+8), kv-heads [2g..2g+2).  Each core
produces a partial [S, D] o-proj contribution; the host sums the 4
partials per batch (the "all-reduce" of row-parallel wo).

Software-pipelined loop over the 4 sequence blocks sb:
  P(sb): QKV projection of s-columns [512*sb, 512*sb+512) in bf16.
         Q^T/K^T produced in [hd, s] layout with RoPE via a signed
         permutation matmul (bf16); V produced in [hd, s] then
         PE-transposed to natural [s, hd].
  A(sb) ∥ O(sb-1): causal attention for q-block sb over keys
         [0, 512*sb+512).  S^T tiles = K^T.T @ Q^T, P^T = exp(S^T*scale)
         on ACT (exp pairs two key tiles; diagonal pairs exp from the
         wider tile's first live column - dead columns are never read);
         the exact-diagonal 128-chunk is masked on GPSIMD.  The softmax
         denominator partial-sums on the DVE in bf16 and folds across
         partitions with one ones-matmul per head.  Because the exp
         stream makes A ACT-bound, the PREVIOUS block's o-proj tiles are
         interleaved as PE filler (4 output tiles per head; one of them
         placed to hide the 1/l reciprocal latency).  attnT is
         double-buffered; wo stays SBUF-resident all run.
  Output partials leave as fp16 (halved out-DMA); host sums the 4 TP
  partials per batch in f32.
All matmul operands are bf16/fp16 (f32 PSUM accumulation); the kernel
is tensor-engine bound at ~90% PE occupancy.
"""

import numpy as np
import ml_dtypes

import concourse.bass as bass
import concourse.tile as tile
from concourse import bacc, mybir
from concourse.bass_utils import run_bass_kernel_spmd

BF16 = mybir.dt.bfloat16
F16 = mybir.dt.float16
F32 = mybir.dt.float32
F32R = mybir.dt.float32r

B, S, D, H, KVH, HD = 2, 2048, 4096, 32, 8, 128
G = 4                      # TP groups
HPG = H // G               # q heads per core = 8
KVPG = KVH // G            # kv heads per core = 2
NW = HPG + 2 * KVPG        # 12 projection "heads" per core (k0,k1,v0,v1,q0-7)
SCALE = 1.0 / float(np.sqrt(HD))
SB = 512                   # s-block (proj free dim, q-block, unit of pipeline)
NSB = S // SB              # 4
DT = D // 128              # 32 contraction tiles
NKT = S // 128             # 16 key tiles
N_CORES = 8

_CACHE: dict = {}


def _build():
    nc = bacc.Bacc("TRN2", target_bir_lowering=False, debug=False,
                   num_devices=N_CORES)

    # ---- DRAM I/O ----
    x_t = nc.dram_tensor("x_t", [NSB, 128, DT, SB], BF16,
                         kind="ExternalInput").ap()
    w_t = nc.dram_tensor("w_t", [NW, 128, DT, 128], BF16,
                         kind="ExternalInput").ap()
    wo_t = nc.dram_tensor("wo_t", [8, 128, HPG, 512], BF16,
                          kind="ExternalInput").ap()
    cosT = nc.dram_tensor("cosT", [NSB, 128, SB], F16, kind="ExternalInput").ap()
    sinT = nc.dram_tensor("sinT", [NSB, 128, SB], F16, kind="ExternalInput").ap()
    permT = nc.dram_tensor("permT", [128, 128], BF16, kind="ExternalInput").ap()
    maskT = nc.dram_tensor("maskT", [128, 128], BF16, kind="ExternalInput").ap()
    ones_col = nc.dram_tensor("ones_col", [128, 1], BF16, kind="ExternalInput").ap()
    ones_row = nc.dram_tensor("ones_row", [1, 128], F16, kind="ExternalInput").ap()
    ident = nc.dram_tensor("ident", [128, 128], BF16, kind="ExternalInput").ap()
    out = nc.dram_tensor("out", [NSB, 8, 4, 128, 512], F16,
                         kind="ExternalOutput").ap()

    with tile.TileContext(nc) as tc:
        with (
            tc.tile_pool(name="pers", bufs=1) as pers,
            tc.tile_pool(name="work", bufs=1) as wk,
            tc.tile_pool(name="psum", bufs=1, space="PSUM") as psum,
        ):
            # long-lived SBUF tensors
            kt_sb = pers.tile([128, KVPG, S], BF16, tag="kt")      # K^T roped
            v_sb = pers.tile([128, NKT, KVPG * 128], BF16, tag="v")  # V natural
            perm_sb = pers.tile([128, 128], BF16, tag="perm")
            mask_sb = pers.tile([128, 128], BF16, tag="mask")
            onec_sb = pers.tile([128, 1], BF16, tag="onec")
            oner_sb = pers.tile([1, 128], F16, tag="oner")
            id_sb = pers.tile([128, 128], BF16, tag="ident")
            wo_all = pers.tile([128, 8, HPG, 512], BF16, tag="wo")  # resident wo
            nc.gpsimd.dma_start(out=id_sb, in_=ident)
            nc.gpsimd.dma_start(out=perm_sb, in_=permT)
            nc.gpsimd.dma_start(out=mask_sb, in_=maskT)
            nc.gpsimd.dma_start(out=onec_sb, in_=ones_col)
            nc.gpsimd.dma_start(out=oner_sb, in_=ones_row)

            def load_rope(sb):
                cb = wk.tile([128, SB], F16, tag="cosb", bufs=2, name="cosb")
                sb_ = wk.tile([128, SB], F16, tag="sinb", bufs=2, name="sinb")
                nc.gpsimd.dma_start(out=cb, in_=cosT[sb])
                nc.gpsimd.dma_start(out=sb_, in_=sinT[sb])
                return cb, sb_

            rope_tbl = load_rope(0)
            # only the first two wo blocks load during the DMA-hungry P(0);
            # the rest are queued behind P(0)'s weight stream on sync
            for dblk in range(2):
                nc.gpsimd.dma_start(out=wo_all[:, dblk], in_=wo_t[dblk])

            def load_xp(sb, chunks=(slice(0, 16), slice(16, 32))):
                xp = wk.tile([128, DT, SB], BF16, tag="xp", bufs=1, name="xp")
                for c in chunks:
                    nc.gpsimd.dma_start(out=xp[:, c, :], in_=x_t[sb, :, c, :])
                return xp

            def load_wh(w_idx, split=1):
                wh = wk.tile([128, DT, 128], BF16, tag="wh", bufs=2, name="wh")
                q = nc.sync if w_idx % 2 else nc.scalar
                n = DT // split
                for hc in range(split):
                    q.dma_start(
                        out=wh[:, hc * n:(hc + 1) * n, :],
                        in_=w_t[w_idx, :, hc * n:(hc + 1) * n, :])
                return wh

            # startup: x chunks stream on sync+vector while the first two
            # weight heads interleave on scalar/sync, so the first proj
            # chain is never starved for either operand
            wh0 = wk.tile([128, DT, 128], BF16, tag="wh", bufs=2, name="wh")
            xp = wk.tile([128, DT, SB], BF16, tag="xp", bufs=1, name="xp")
            nc.scalar.dma_start(out=wh0[:, 0:4, :], in_=w_t[0, :, 0:4, :])
            nc.sync.dma_start(out=xp[:, 0:4, :], in_=x_t[0, :, 0:4, :])
            nc.scalar.dma_start(out=wh0[:, 4:16, :], in_=w_t[0, :, 4:16, :])
            nc.sync.dma_start(out=xp[:, 4:12, :], in_=x_t[0, :, 4:12, :])
            nc.scalar.dma_start(out=wh0[:, 16:32, :], in_=w_t[0, :, 16:32, :])
            for c in range(3, 8):
                cs = slice(c * 4, (c + 1) * 4)
                nc.gpsimd.dma_start(out=xp[:, cs, :], in_=x_t[0, :, cs, :])
            wh1 = load_wh(1)
            pending_wh = {0: wh0, 1: wh1}
            o_work = []
            for sb in range(NSB):
                scols = slice(sb * SB, (sb + 1) * SB)
                cos_blk, sin_blk = rope_tbl
                if sb + 1 < NSB:
                    rope_tbl = load_rope(sb + 1)

                # ============ P(sb): QKV projection + RoPE ============
                rope_pending = []

                def flush_rope_one():
                    raw, dst = rope_pending.pop(0)
                    pp = psum.tile([128, SB], F32, tag="mm", bufs=2,
                                   padded_shape=[128, SB * 2], name="pp")
                    nc.tensor.matmul(pp, perm_sb, raw, start=True, stop=True)
                    nc.vector.tensor_mul(dst, pp, sin_blk)
                    nc.vector.tensor_add(dst, dst, t1s.pop(id(raw)))

                def flush_rope():
                    while rope_pending:
                        flush_rope_one()

                t1s = {}
                for w_idx in range(NW):
                    wh = pending_wh.pop(w_idx, None)
                    if wh is None:
                        wh = load_wh(w_idx)
                    if 2 <= w_idx < 4:
                        # v head: [hd, s] proj then PE-transpose to natural
                        kvs = w_idx - 2
                        acc = psum.tile([128, SB], F32, tag="acc", bufs=2,
                                        name="acc")
                        for dt_i in range(DT):
                            nc.tensor.matmul(
                                acc, wh[:, dt_i, :], xp[:, dt_i, :],
                                start=(dt_i == 0), stop=(dt_i == DT - 1))
                        vtp = wk.tile([128, SB], BF16, tag="vtp", bufs=1)
                        nc.vector.tensor_copy(vtp, acc)
                        for blk in range(SB // 128):
                            tp = psum.tile([128, 128], BF16, tag="mm", bufs=2,
                                           padded_shape=[128, SB * 4], name="tp")
                            nc.tensor.transpose(
                                tp, vtp[:, blk * 128:(blk + 1) * 128], id_sb)
                            nc.vector.tensor_copy(
                                v_sb[:, sb * 4 + blk,
                                     kvs * 128:(kvs + 1) * 128], tp)
                    else:
                        # q or k head: [hd, s] layout + rope
                        acc = psum.tile([128, SB], F32, tag="acc", bufs=2)
                        for dt_i in range(DT):
                            nc.tensor.matmul(
                                acc, wh[:, dt_i, :], xp[:, dt_i, :],
                                start=(dt_i == 0), stop=(dt_i == DT - 1))
                        raw = wk.tile([128, SB], BF16, tag="raw", bufs=2)
                        nc.vector.tensor_copy(raw, acc)
                        t1 = wk.tile([128, SB], F32, tag="t1", bufs=2)
                        nc.vector.tensor_mul(t1, raw, cos_blk)
                        t1s[id(raw)] = t1
                        if w_idx < 2:
                            dst = kt_sb[:, w_idx, scols]
                        else:
                            if w_idx == 4:
                                qt = wk.tile([128, HPG, SB], BF16, tag="qt",
                                             bufs=1)
                            dst = qt[:, w_idx - 4, :]
                        rope_pending.append((raw, dst))
                        if len(rope_pending) > 1:
                            flush_rope_one()

                flush_rope()

                if sb == 0:
                    # remaining wo blocks, queued after P(0)'s weight stream
                    for dblk in range(2, 8):
                        q = nc.sync if dblk % 2 else nc.scalar
                        q.dma_start(out=wo_all[:, dblk], in_=wo_t[dblk])

                # prefetch next s-block activations; the DMA starts as soon
                # as P(sb)'s last read of the single xp buffer retires and
                # hides under A(sb)+O(sb)
                if sb + 1 < NSB:
                    xp = load_xp(sb + 1)

                # ============ A(sb): attention q-block qi=sb ============
                # Scores/exp for head h+1 are interleaved (cross-head
                # software pipeline) with PV/l-row of head h so the tensor
                # engine never idles waiting on the ACT exp stream (idle
                # gaps re-engage the HAM clock throttle).
                nkt = 4 * sb + 4

                def q0(kti):  # first causally-live query column for key tile
                    return 128 * (kti - 4 * sb) if kti >= 4 * sb else 0

                # every task is a pair of key tiles sharing one wide exp;
                # diagonal pairs exp from the wider tile's first live column
                # (the partner's dead columns are never read downstream)
                tasks = []
                for h in range(HPG):
                    tasks += [(h, (k, k + 1)) for k in range(0, nkt, 2)]
                pt_tiles = {}
                state = {"ti": 0, "tiles": 0}

                def issue_task():
                    if state["ti"] >= len(tasks):
                        return
                    h2, ks = tasks[state["ti"]]
                    state["ti"] += 1
                    state["tiles"] += 2
                    if ks[0] == 0:
                        pt_tiles[h2] = wk.tile([128, NKT, SB], BF16, tag="pt",
                                               bufs=2, name="pt")
                    pt = pt_tiles[h2]
                    kvs2 = h2 // (HPG // KVPG)
                    st = psum.tile([128, 2, SB], F32, tag="mm", bufs=2,
                                   name="st")
                    for j, k in enumerate(ks):
                        qo = q0(k)
                        nc.tensor.matmul(
                            st[:, j, qo:],
                            kt_sb[:, kvs2, k * 128:(k + 1) * 128],
                            qt[:, h2, qo:],
                            start=True, stop=True)
                    eqo = q0(ks[0])
                    nc.scalar.activation(
                        pt[:, ks[0]:ks[0] + 2, eqo:], st[:, :, eqo:],
                        mybir.ActivationFunctionType.Exp, scale=SCALE)
                    for k in ks:
                        if k >= 4 * sb:  # diagonal tile: mask its 128-chunk
                            qo = q0(k)
                            nc.gpsimd.tensor_mul(
                                pt[:, k, qo:qo + 128],
                                pt[:, k, qo:qo + 128], mask_sb)

                def emit_o_tile(at_prev, psb, dblk, qs):
                    # one o-proj output tile of s-block psb (PE filler work)
                    ops = psum.tile([128, 512], F32, tag="alt", bufs=2,
                                    name="ops")
                    for hp in range(HPG):
                        nc.tensor.matmul(
                            ops,
                            at_prev[:, hp, qs * 128:(qs + 1) * 128],
                            wo_all[:, dblk, hp, :],
                            start=(hp == 0), stop=(hp == HPG - 1))
                    o_sb = wk.tile([128, 512], F16, tag="osb", bufs=4)
                    nc.vector.tensor_copy(o_sb, ops)
                    nc.scalar.dma_start(out=out[psb, dblk, qs], in_=o_sb)

                while state["tiles"] < 4 and state["ti"] < len(tasks):
                    issue_task()
                attnT = wk.tile([128, HPG, SB], BF16, tag="attnT", bufs=2)
                consumed = 0
                for h in range(HPG):
                    kvs = h // (HPG // KVPG)
                    pt = pt_tiles[h]
                    oT = psum.tile([128, SB], F32, tag="acc", bufs=2)
                    lrow = psum.tile([1, SB], F32, tag="alt", bufs=2,
                                     name="lrow")
                    # softmax denominator: partial-sum all key tiles on the
                    # DVE in bf16 (2x rate), fold across partitions with one
                    # ones-matmul per head
                    lp = wk.tile([128, SB], BF16, tag="lp", bufs=1)
                    for kti in range(nkt):
                        qo = q0(kti)
                        nc.tensor.matmul(
                            oT[:, qo:],
                            v_sb[:, kti, kvs * 128:(kvs + 1) * 128],
                            pt[:, kti, qo:],
                            start=(kti == 0), stop=(kti == nkt - 1))
                        if kti == 0:
                            nc.vector.tensor_copy(lp, pt[:, 0, :])
                        else:
                            nc.vector.tensor_add(
                                lp[:, qo:], lp[:, qo:], pt[:, kti, qo:])
                        consumed += 1
                        while (state["tiles"] < consumed + 4
                               and state["ti"] < len(tasks)):
                            issue_task()
                    nc.tensor.matmul(lrow, onec_sb, lp,
                                     start=True, stop=True)
                    # finalize: attnT = oT * bcast(1/l).  One o-proj filler
                    # tile of the previous s-block hides the DVE reciprocal
                    # latency between the fold and bc matmuls; three more
                    # keep the PE fed while the ACT exp stream catches up.
                    # (recip is issued BEFORE the filler so the filler's DVE
                    # drain doesn't delay it on the in-order DVE queue)
                    linv = wk.tile([1, SB], F32, tag="linv", bufs=1)
                    nc.vector.reciprocal_approx_fast(linv, lrow)
                    linv_r = wk.tile([1, SB], F16, tag="linvr", bufs=1)
                    nc.vector.tensor_copy(linv_r, linv)
                    if o_work:
                        emit_o_tile(*o_work.pop(0))
                    bc = psum.tile([128, SB], F32, tag="alt", bufs=2,
                                   name="bc")
                    nc.tensor.matmul(bc, oner_sb, linv_r,
                                     start=True, stop=True)
                    bc_sb = wk.tile([128, SB], F16, tag="bcsb", bufs=1)
                    nc.vector.tensor_copy(bc_sb, bc)
                    nc.vector.tensor_mul(attnT[:, h, :], oT, bc_sb)
                    for _ in range(3):
                        if o_work:
                            emit_o_tile(*o_work.pop(0))

                # o-proj of THIS s-block: deferred into A(sb+1) as PE filler
                # (the ACT-bound exp stream there leaves PE slack); the last
                # block's runs right here
                while o_work:
                    emit_o_tile(*o_work.pop(0))
                o_work = [(attnT, sb, dblk, qs)
                          for dblk in range(8) for qs in range(4)]
                if sb + 1 == NSB:
                    while o_work:
                        emit_o_tile(*o_work.pop(0))

                # prefetch next s-block weights ahead of the out-DMA backlog
                if sb + 1 < NSB:
                    pending_wh = {0: load_wh(0), 1: load_wh(1)}
    nc.compile()
    return nc


def _host_inputs(x, wq, wk, wv, wo, cos, sin):
    """Build the 8 per-core input maps (all host-side prep)."""
    x = np.asarray(x, np.float32)
    wq = np.asarray(wq, np.float32)
    wk = np.asarray(wk, np.float32)
    wv = np.asarray(wv, np.float32)
    wo = np.asarray(wo, np.float32)
    cos = np.asarray(cos, np.float32)
    sin = np.asarray(sin, np.float32)

    # [NSB, 128, SB] fp16 per-block rotary tables (rows repeated in pairs)
    cosT = np.ascontiguousarray(
        np.repeat(cos.T, 2, axis=0).reshape(128, NSB, SB).transpose(1, 0, 2)
    ).astype(np.float16)
    sinT = np.ascontiguousarray(
        np.repeat(sin.T, 2, axis=0).reshape(128, NSB, SB).transpose(1, 0, 2)
    ).astype(np.float16)
    permT = np.zeros((128, 128), ml_dtypes.bfloat16)
    idx = np.arange(64)
    permT[2 * idx + 1, 2 * idx] = -1.0
    permT[2 * idx, 2 * idx + 1] = 1.0
    kk = np.arange(128)[:, None]
    tt = np.arange(128)[None, :]
    maskT = (tt >= kk).astype(ml_dtypes.bfloat16)      # [128, 128]
    ones_col = np.ones((128, 1), ml_dtypes.bfloat16)
    ones_row = np.ones((1, 128), np.float16)
    ident = np.eye(128, dtype=ml_dtypes.bfloat16)

    def tile_w(w_col):  # [D, 128] -> [128, DT, 128]
        return w_col.reshape(DT, 128, 128).transpose(1, 0, 2)

    x_ts = [np.ascontiguousarray(
        x[b].T.reshape(DT, 128, NSB, SB).transpose(2, 1, 0, 3)
    ).astype(ml_dtypes.bfloat16) for b in range(B)]
    in_maps = []
    for core in range(N_CORES):
        b, g = divmod(core, G)
        w_np = np.empty((NW, 128, DT, 128), np.float32)
        for j in range(KVPG):
            w_np[j] = tile_w(wk[:, (g * KVPG + j) * 128:(g * KVPG + j + 1) * 128])
        for j in range(KVPG):
            w_np[KVPG + j] = tile_w(
                wv[:, (g * KVPG + j) * 128:(g * KVPG + j + 1) * 128])
        for j in range(HPG):
            w_np[2 * KVPG + j] = tile_w(
                wq[:, (g * HPG + j) * 128:(g * HPG + j + 1) * 128])
        wo_g = wo[g * HPG * HD:(g + 1) * HPG * HD, :]          # [1024, D]
        wo_np = np.ascontiguousarray(
            wo_g.reshape(HPG, 128, 8, 512).transpose(2, 1, 0, 3)
        ).astype(ml_dtypes.bfloat16)                           # [8, 128, HPG, 512]
        in_maps.append({
            "x_t": x_ts[b], "w_t": w_np.astype(ml_dtypes.bfloat16),
            "wo_t": wo_np,
            "cosT": cosT, "sinT": sinT, "permT": permT, "maskT": maskT,
            "ones_col": ones_col, "ones_row": ones_row, "ident": ident,
        })
    return in_maps


def kernel(x, wq, wk, wv, wo, cos, sin, mask, start_pos):
    assert int(start_pos) == 0, "kernel compiled for prefill (start_pos=0)"
    if "nc" not in _CACHE:
        _CACHE["nc"] = _build()
    nc = _CACHE["nc"]
    in_maps = _host_inputs(x, wq, wk, wv, wo, cos, sin)
    res = run_bass_kernel_spmd(nc, in_maps, list(range(N_CORES)))

    def unpack(o):  # [NSB, 8, 4, 128, 512] -> [S, D]
        return np.ascontiguousarray(
            np.transpose(o, (0, 2, 3, 1, 4)).reshape(S, D))

    full = np.empty((B, S, D), np.float32)
    for b in range(B):
        acc = res.results[4 * b]["out"].astype(np.float32)
        for g in range(1, G):
            acc = acc + res.results[4 * b + g]["out"]
        full[b] = unpack(acc)
    return full



# revision 34
# speedup vs baseline: 1.0031x; 1.0031x over previous
"""Trainium2 Bass kernel for GQA attention layer (Llama-style, prefill).

Full computation:  out = softmax((rope(x@wq) @ rope(x@wk)^T)*scale + causal) @ (x@wv) @ wo

Sharding: 8 cores = DP(2 batches) x TP(4 head-groups).  Core c = 4*b + g
handles batch b, q-heads [8g..8g+8), kv-heads [2g..2g+2).  Each core
produces a partial [S, D] o-proj contribution; the host sums the 4
partials per batch (the "all-reduce" of row-parallel wo).

Software-pipelined loop over the 4 sequence blocks sb:
  P(sb): QKV projection of s-columns [512*sb, 512*sb+512) in bf16.
         Q^T/K^T produced in [hd, s] layout with RoPE via a signed
         permutation matmul (bf16); V produced in [hd, s] then
         PE-transposed to natural [s, hd].
  A(sb) ∥ O(sb-1): causal attention for q-block sb over keys
         [0, 512*sb+512).  S^T tiles = K^T.T @ Q^T, P^T = exp(S^T*scale)
         on ACT (exp pairs two key tiles; diagonal pairs exp from the
         wider tile's first live column - dead columns are never read);
         the exact-diagonal 128-chunk is masked on GPSIMD.  The softmax
         denominator partial-sums on the DVE in bf16 and folds across
         partitions with one ones-matmul per head.  Because the exp
         stream makes A ACT-bound, the PREVIOUS block's o-proj tiles are
         interleaved as PE filler (4 output tiles per head; one of them
         placed to hide the 1/l reciprocal latency).  attnT is
         double-buffered; wo stays SBUF-resident all run.
  Output partials leave as fp16 (halved out-DMA); host sums the 4 TP
  partials per batch in f32.
All matmul operands are bf16/fp16 (f32 PSUM accumulation); the kernel
is tensor-engine bound at ~90% PE occupancy.
"""

import numpy as np
import ml_dtypes

import concourse.bass as bass
import concourse.tile as tile
from concourse import bacc, mybir
from concourse.bass_utils import run_bass_kernel_spmd

BF16 = mybir.dt.bfloat16
F16 = mybir.dt.float16
F32 = mybir.dt.float32
F32R = mybir.dt.float32r

B, S, D, H, KVH, HD = 2, 2048, 4096, 32, 8, 128
G = 4                      # TP groups
HPG = H // G               # q heads per core = 8
KVPG = KVH // G            # kv heads per core = 2
NW = HPG + 2 * KVPG        # 12 projection "heads" per core (k0,k1,v0,v1,q0-7)
SCALE = 1.0 / float(np.sqrt(HD))
SB = 512                   # s-block (proj free dim, q-block, unit of pipeline)
NSB = S // SB              # 4
DT = D // 128              # 32 contraction tiles
NKT = S // 128             # 16 key tiles
N_CORES = 8

_CACHE: dict = {}


def _build():
    nc = bacc.Bacc("TRN2", target_bir_lowering=False, debug=False,
                   num_devices=N_CORES)

    # ---- DRAM I/O ----
    x_t = nc.dram_tensor("x_t", [NSB, 128, DT, SB], BF16,
                         kind="ExternalInput").ap()
    w_t = nc.dram_tensor("w_t", [NW, 128, DT, 128], BF16,
                         kind="ExternalInput").ap()
    wo_t = nc.dram_tensor("wo_t", [8, 128, HPG, 512], BF16,
                          kind="ExternalInput").ap()
    cosT = nc.dram_tensor("cosT", [NSB, 128, SB], F16, kind="ExternalInput").ap()
    sinT = nc.dram_tensor("sinT", [NSB, 128, SB], F16, kind="ExternalInput").ap()
    permT = nc.dram_tensor("permT", [128, 128], BF16, kind="ExternalInput").ap()
    maskT = nc.dram_tensor("maskT", [128, 128], BF16, kind="ExternalInput").ap()
    ones_col = nc.dram_tensor("ones_col", [128, 1], BF16, kind="ExternalInput").ap()
    ones_row = nc.dram_tensor("ones_row", [1, 128], F16, kind="ExternalInput").ap()
    ident = nc.dram_tensor("ident", [128, 128], BF16, kind="ExternalInput").ap()
    out = nc.dram_tensor("out", [NSB, 8, 4, 128, 512], F16,
                         kind="ExternalOutput").ap()

    with tile.TileContext(nc) as tc:
        with (
            tc.tile_pool(name="pers", bufs=1) as pers,
            tc.tile_pool(name="work", bufs=1) as wk,
            tc.tile_pool(name="psum", bufs=1, space="PSUM") as psum,
        ):
            # long-lived SBUF tensors
            kt_sb = pers.tile([128, KVPG, S], BF16, tag="kt")      # K^T roped
            v_sb = pers.tile([128, NKT, KVPG * 128], BF16, tag="v")  # V natural
            perm_sb = pers.tile([128, 128], BF16, tag="perm")
            mask_sb = pers.tile([128, 128], BF16, tag="mask")
            onec_sb = pers.tile([128, 1], BF16, tag="onec")
            oner_sb = pers.tile([1, 128], F16, tag="oner")
            id_sb = pers.tile([128, 128], BF16, tag="ident")
            wo_all = pers.tile([128, 8, HPG, 512], BF16, tag="wo")  # resident wo
            nc.gpsimd.dma_start(out=id_sb, in_=ident)
            nc.gpsimd.dma_start(out=perm_sb, in_=permT)
            nc.gpsimd.dma_start(out=mask_sb, in_=maskT)
            nc.gpsimd.dma_start(out=onec_sb, in_=ones_col)
            nc.gpsimd.dma_start(out=oner_sb, in_=ones_row)

            def load_rope(sb):
                cb = wk.tile([128, SB], F16, tag="cosb", bufs=2, name="cosb")
                sb_ = wk.tile([128, SB], F16, tag="sinb", bufs=2, name="sinb")
                nc.gpsimd.dma_start(out=cb, in_=cosT[sb])
                nc.gpsimd.dma_start(out=sb_, in_=sinT[sb])
                return cb, sb_

            rope_tbl = load_rope(0)

            def load_xp(sb, chunks=(slice(0, 16), slice(16, 32))):
                xp = wk.tile([128, DT, SB], BF16, tag="xp", bufs=1, name="xp")
                for c in chunks:
                    nc.gpsimd.dma_start(out=xp[:, c, :], in_=x_t[sb, :, c, :])
                return xp

            def load_wh(w_idx, split=1):
                wh = wk.tile([128, DT, 128], BF16, tag="wh", bufs=2, name="wh")
                q = nc.sync if w_idx % 2 else nc.scalar
                n = DT // split
                for hc in range(split):
                    q.dma_start(
                        out=wh[:, hc * n:(hc + 1) * n, :],
                        in_=w_t[w_idx, :, hc * n:(hc + 1) * n, :])
                return wh

            # startup: sync is dedicated to the x panel (the first proj
            # chains sweep all of it), wh0 on scalar, wh1 on gpsimd; the
            # first two wo blocks queue behind everything critical
            wh0 = wk.tile([128, DT, 128], BF16, tag="wh", bufs=2, name="wh")
            wh1 = wk.tile([128, DT, 128], BF16, tag="wh", bufs=2, name="wh")
            xp = wk.tile([128, DT, SB], BF16, tag="xp", bufs=1, name="xp")
            nc.scalar.dma_start(out=wh0[:, 0:4, :], in_=w_t[0, :, 0:4, :])
            nc.sync.dma_start(out=xp[:, 0:4, :], in_=x_t[0, :, 0:4, :])
            nc.scalar.dma_start(out=wh0[:, 4:16, :], in_=w_t[0, :, 4:16, :])
            nc.sync.dma_start(out=xp[:, 4:8, :], in_=x_t[0, :, 4:8, :])
            nc.scalar.dma_start(out=wh0[:, 16:32, :], in_=w_t[0, :, 16:32, :])
            nc.gpsimd.dma_start(out=wh1[:, 0:8, :], in_=w_t[1, :, 0:8, :])
            nc.sync.dma_start(out=xp[:, 8:16, :], in_=x_t[0, :, 8:16, :])
            nc.gpsimd.dma_start(out=wh1[:, 8:32, :], in_=w_t[1, :, 8:32, :])
            nc.sync.dma_start(out=xp[:, 16:24, :], in_=x_t[0, :, 16:24, :])
            nc.sync.dma_start(out=xp[:, 24:32, :], in_=x_t[0, :, 24:32, :])
            for dblk in range(2):
                nc.gpsimd.dma_start(out=wo_all[:, dblk], in_=wo_t[dblk])
            pending_wh = {0: wh0, 1: wh1}
            o_work = []
            for sb in range(NSB):
                scols = slice(sb * SB, (sb + 1) * SB)
                cos_blk, sin_blk = rope_tbl
                if sb + 1 < NSB:
                    rope_tbl = load_rope(sb + 1)

                # ============ P(sb): QKV projection + RoPE ============
                rope_pending = []

                def flush_rope_one():
                    raw, dst = rope_pending.pop(0)
                    pp = psum.tile([128, SB], F32, tag="mm", bufs=2,
                                   padded_shape=[128, SB * 2], name="pp")
                    nc.tensor.matmul(pp, perm_sb, raw, start=True, stop=True)
                    nc.vector.tensor_mul(dst, pp, sin_blk)
                    nc.vector.tensor_add(dst, dst, t1s.pop(id(raw)))

                def flush_rope():
                    while rope_pending:
                        flush_rope_one()

                t1s = {}

                def rope_head(acc, w_idx):
                    # PSUM -> raw/t1, queue the rope flush for this head
                    raw = wk.tile([128, SB], BF16, tag="raw", bufs=2)
                    nc.vector.tensor_copy(raw, acc)
                    t1 = wk.tile([128, SB], F32, tag="t1", bufs=2)
                    nc.vector.tensor_mul(t1, raw, cos_blk)
                    t1s[id(raw)] = t1
                    if w_idx < 2:
                        dst = kt_sb[:, w_idx, scols]
                    else:
                        dst = qt[:, w_idx - 4, :]
                    rope_pending.append((raw, dst))
                    if len(rope_pending) > 1:
                        flush_rope_one()

                if sb == 0:
                    # the first chain is DMA-throttled (one head sweeps the
                    # x panel at ~2x fabric rate); interleaving the two
                    # k-heads halves the per-chunk demand rate
                    whA = pending_wh.pop(0)
                    whB = pending_wh.pop(1)
                    accA = psum.tile([128, SB], F32, tag="acc", bufs=2,
                                     name="acc")
                    accB = psum.tile([128, SB], F32, tag="acc", bufs=2,
                                     name="acc")
                    for dt_i in range(DT):
                        nc.tensor.matmul(
                            accA, whA[:, dt_i, :], xp[:, dt_i, :],
                            start=(dt_i == 0), stop=(dt_i == DT - 1))
                        nc.tensor.matmul(
                            accB, whB[:, dt_i, :], xp[:, dt_i, :],
                            start=(dt_i == 0), stop=(dt_i == DT - 1))
                    rope_head(accA, 0)
                    rope_head(accB, 1)
                    head_list = range(2, NW)
                else:
                    head_list = range(NW)

                for w_idx in head_list:
                    wh = pending_wh.pop(w_idx, None)
                    if wh is None:
                        wh = load_wh(w_idx)
                    if 2 <= w_idx < 4:
                        # v head: [hd, s] proj then PE-transpose to natural
                        kvs = w_idx - 2
                        acc = psum.tile([128, SB], F32, tag="acc", bufs=2,
                                        name="acc")
                        for dt_i in range(DT):
                            nc.tensor.matmul(
                                acc, wh[:, dt_i, :], xp[:, dt_i, :],
                                start=(dt_i == 0), stop=(dt_i == DT - 1))
                        vtp = wk.tile([128, SB], BF16, tag="vtp", bufs=1)
                        nc.vector.tensor_copy(vtp, acc)
                        for blk in range(SB // 128):
                            tp = psum.tile([128, 128], BF16, tag="mm", bufs=2,
                                           padded_shape=[128, SB * 4], name="tp")
                            nc.tensor.transpose(
                                tp, vtp[:, blk * 128:(blk + 1) * 128], id_sb)
                            nc.vector.tensor_copy(
                                v_sb[:, sb * 4 + blk,
                                     kvs * 128:(kvs + 1) * 128], tp)
                    else:
                        # q or k head: [hd, s] layout + rope
                        acc = psum.tile([128, SB], F32, tag="acc", bufs=2)
                        for dt_i in range(DT):
                            nc.tensor.matmul(
                                acc, wh[:, dt_i, :], xp[:, dt_i, :],
                                start=(dt_i == 0), stop=(dt_i == DT - 1))
                        if w_idx == 4:
                            qt = wk.tile([128, HPG, SB], BF16, tag="qt",
                                         bufs=1)
                        rope_head(acc, w_idx)

                flush_rope()

                if sb == 0:
                    # remaining wo blocks, queued after P(0)'s weight stream
                    for dblk in range(2, 8):
                        q = nc.sync if dblk % 2 else nc.scalar
                        q.dma_start(out=wo_all[:, dblk], in_=wo_t[dblk])

                # prefetch next s-block activations; the DMA starts as soon
                # as P(sb)'s last read of the single xp buffer retires and
                # hides under A(sb)+O(sb)
                if sb + 1 < NSB:
                    xp = load_xp(sb + 1)

                # ============ A(sb): attention q-block qi=sb ============
                # Scores/exp for head h+1 are interleaved (cross-head
                # software pipeline) with PV/l-row of head h so the tensor
                # engine never idles waiting on the ACT exp stream (idle
                # gaps re-engage the HAM clock throttle).
                nkt = 4 * sb + 4

                def q0(kti):  # first causally-live query column for key tile
                    return 128 * (kti - 4 * sb) if kti >= 4 * sb else 0

                # every task is a pair of key tiles sharing one wide exp;
                # diagonal pairs exp from the wider tile's first live column
                # (the partner's dead columns are never read downstream)
                tasks = []
                for h in range(HPG):
                    tasks += [(h, (k, k + 1)) for k in range(0, nkt, 2)]
                pt_tiles = {}
                state = {"ti": 0, "tiles": 0}

                def issue_task():
                    if state["ti"] >= len(tasks):
                        return
                    h2, ks = tasks[state["ti"]]
                    state["ti"] += 1
                    state["tiles"] += 2
                    if ks[0] == 0:
                        pt_tiles[h2] = wk.tile([128, NKT, SB], BF16, tag="pt",
                                               bufs=2, name="pt")
                    pt = pt_tiles[h2]
                    kvs2 = h2 // (HPG // KVPG)
                    st = psum.tile([128, 2, SB], F32, tag="mm", bufs=2,
                                   name="st")
                    for j, k in enumerate(ks):
                        qo = q0(k)
                        nc.tensor.matmul(
                            st[:, j, qo:],
                            kt_sb[:, kvs2, k * 128:(k + 1) * 128],
                            qt[:, h2, qo:],
                            start=True, stop=True)
                    eqo = q0(ks[0])
                    nc.scalar.activation(
                        pt[:, ks[0]:ks[0] + 2, eqo:], st[:, :, eqo:],
                        mybir.ActivationFunctionType.Exp, scale=SCALE)
                    for k in ks:
                        if k >= 4 * sb:  # diagonal tile: mask its 128-chunk
                            qo = q0(k)
                            nc.gpsimd.tensor_mul(
                                pt[:, k, qo:qo + 128],
                                pt[:, k, qo:qo + 128], mask_sb)

                def emit_o_tile(at_prev, psb, dblk, qs):
                    # one o-proj output tile of s-block psb (PE filler work)
                    ops = psum.tile([128, 512], F32, tag="alt", bufs=2,
                                    name="ops")
                    for hp in range(HPG):
                        nc.tensor.matmul(
                            ops,
                            at_prev[:, hp, qs * 128:(qs + 1) * 128],
                            wo_all[:, dblk, hp, :],
                            start=(hp == 0), stop=(hp == HPG - 1))
                    o_sb = wk.tile([128, 512], F16, tag="osb", bufs=4)
                    nc.vector.tensor_copy(o_sb, ops)
                    nc.scalar.dma_start(out=out[psb, dblk, qs], in_=o_sb)

                while state["tiles"] < 4 and state["ti"] < len(tasks):
                    issue_task()
                attnT = wk.tile([128, HPG, SB], BF16, tag="attnT", bufs=2)
                consumed = 0
                for h in range(HPG):
                    kvs = h // (HPG // KVPG)
                    pt = pt_tiles[h]
                    oT = psum.tile([128, SB], F32, tag="acc", bufs=2)
                    lrow = psum.tile([1, SB], F32, tag="alt", bufs=2,
                                     name="lrow")
                    # softmax denominator: partial-sum all key tiles on the
                    # DVE in bf16 (2x rate), fold across partitions with one
                    # ones-matmul per head
                    lp = wk.tile([128, SB], BF16, tag="lp", bufs=1)
                    for kti in range(nkt):
                        qo = q0(kti)
                        nc.tensor.matmul(
                            oT[:, qo:],
                            v_sb[:, kti, kvs * 128:(kvs + 1) * 128],
                            pt[:, kti, qo:],
                            start=(kti == 0), stop=(kti == nkt - 1))
                        if kti == 0:
                            nc.vector.tensor_copy(lp, pt[:, 0, :])
                        else:
                            nc.vector.tensor_add(
                                lp[:, qo:], lp[:, qo:], pt[:, kti, qo:])
                        consumed += 1
                        while (state["tiles"] < consumed + 4
                               and state["ti"] < len(tasks)):
                            issue_task()
                    nc.tensor.matmul(lrow, onec_sb, lp,
                                     start=True, stop=True)
                    # finalize: attnT = oT * bcast(1/l).  One o-proj filler
                    # tile of the previous s-block hides the DVE reciprocal
                    # latency between the fold and bc matmuls; three more
                    # keep the PE fed while the ACT exp stream catches up.
                    # (recip is issued BEFORE the filler so the filler's DVE
                    # drain doesn't delay it on the in-order DVE queue)
                    linv = wk.tile([1, SB], F32, tag="linv", bufs=1)
                    nc.vector.reciprocal_approx_fast(linv, lrow)
                    linv_r = wk.tile([1, SB], F16, tag="linvr", bufs=1)
                    nc.vector.tensor_copy(linv_r, linv)
                    if o_work:
                        emit_o_tile(*o_work.pop(0))
                    bc = psum.tile([128, SB], F32, tag="alt", bufs=2,
                                   name="bc")
                    nc.tensor.matmul(bc, oner_sb, linv_r,
                                     start=True, stop=True)
                    bc_sb = wk.tile([128, SB], F16, tag="bcsb", bufs=1)
                    nc.vector.tensor_copy(bc_sb, bc)
                    nc.vector.tensor_mul(attnT[:, h, :], oT, bc_sb)
                    for _ in range(3):
                        if o_work:
                            emit_o_tile(*o_work.pop(0))

                # o-proj of THIS s-block: deferred into A(sb+1) as PE filler
                # (the ACT-bound exp stream there leaves PE slack); the last
                # block's runs right here
                while o_work:
                    emit_o_tile(*o_work.pop(0))
                o_work = [(attnT, sb, dblk, qs)
                          for dblk in range(8) for qs in range(4)]
                if sb + 1 == NSB:
                    while o_work:
                        emit_o_tile(*o_work.pop(0))

                # prefetch next s-block weights ahead of the out-DMA backlog
                if sb + 1 < NSB:
                    pending_wh = {0: load_wh(0), 1: load_wh(1)}
    nc.compile()
    return nc


def _host_inputs(x, wq, wk, wv, wo, cos, sin):
    """Build the 8 per-core input maps (all host-side prep)."""
    x = np.asarray(x, np.float32)
    wq = np.asarray(wq, np.float32)
    wk = np.asarray(wk, np.float32)
    wv = np.asarray(wv, np.float32)
    wo = np.asarray(wo, np.float32)
    cos = np.asarray(cos, np.float32)
    sin = np.asarray(sin, np.float32)

    # [NSB, 128, SB] fp16 per-block rotary tables (rows repeated in pairs)
    cosT = np.ascontiguousarray(
        np.repeat(cos.T, 2, axis=0).reshape(128, NSB, SB).transpose(1, 0, 2)
    ).astype(np.float16)
    sinT = np.ascontiguousarray(
        np.repeat(sin.T, 2, axis=0).reshape(128, NSB, SB).transpose(1, 0, 2)
    ).astype(np.float16)
    permT = np.zeros((128, 128), ml_dtypes.bfloat16)
    idx = np.arange(64)
    permT[2 * idx + 1, 2 * idx] = -1.0
    permT[2 * idx, 2 * idx + 1] = 1.0
    kk = np.arange(128)[:, None]
    tt = np.arange(128)[None, :]
    maskT = (tt >= kk).astype(ml_dtypes.bfloat16)      # [128, 128]
    ones_col = np.ones((128, 1), ml_dtypes.bfloat16)
    ones_row = np.ones((1, 128), np.float16)
    ident = np.eye(128, dtype=ml_dtypes.bfloat16)

    def tile_w(w_col):  # [D, 128] -> [128, DT, 128]
        return w_col.reshape(DT, 128, 128).transpose(1, 0, 2)

    x_ts = [np.ascontiguousarray(
        x[b].T.reshape(DT, 128, NSB, SB).transpose(2, 1, 0, 3)
    ).astype(ml_dtypes.bfloat16) for b in range(B)]
    in_maps = []
    for core in range(N_CORES):
        b, g = divmod(core, G)
        w_np = np.empty((NW, 128, DT, 128), np.float32)
        for j in range(KVPG):
            w_np[j] = tile_w(wk[:, (g * KVPG + j) * 128:(g * KVPG + j + 1) * 128])
        for j in range(KVPG):
            w_np[KVPG + j] = tile_w(
                wv[:, (g * KVPG + j) * 128:(g * KVPG + j + 1) * 128])
        for j in range(HPG):
            w_np[2 * KVPG + j] = tile_w(
                wq[:, (g * HPG + j) * 128:(g * HPG + j + 1) * 128])
        wo_g = wo[g * HPG * HD:(g + 1) * HPG * HD, :]          # [1024, D]
        wo_np = np.ascontiguousarray(
            wo_g.reshape(HPG, 128, 8, 512).transpose(2, 1, 0, 3)
        ).astype(ml_dtypes.bfloat16)                           # [8, 128, HPG, 512]
        in_maps.append({
            "x_t": x_ts[b], "w_t": w_np.astype(ml_dtypes.bfloat16),
            "wo_t": wo_np,
            "cosT": cosT, "sinT": sinT, "permT": permT, "maskT": maskT,
            "ones_col": ones_col, "ones_row": ones_row, "ident": ident,
        })
    return in_maps


def kernel(x, wq, wk, wv, wo, cos, sin, mask, start_pos):
    assert int(start_pos) == 0, "kernel compiled for prefill (start_pos=0)"
    if "nc" not in _CACHE:
        _CACHE["nc"] = _build()
    nc = _CACHE["nc"]
    in_maps = _host_inputs(x, wq, wk, wv, wo, cos, sin)
    res = run_bass_kernel_spmd(nc, in_maps, list(range(N_CORES)))

    def unpack(o):  # [NSB, 8, 4, 128, 512] -> [S, D]
        return np.ascontiguousarray(
            np.transpose(o, (0, 2, 3, 1, 4)).reshape(S, D))

    full = np.empty((B, S, D), np.float32)
    for b in range(B):
        acc = res.results[4 * b]["out"].astype(np.float32)
        for g in range(1, G):
            acc = acc + res.results[4 * b + g]["out"]
        full[b] = unpack(acc)
    return full



# revision 35
# speedup vs baseline: 1.0105x; 1.0074x over previous
"""Trainium2 Bass kernel for GQA attention layer (Llama-style, prefill).

Full computation:  out = softmax((rope(x@wq) @ rope(x@wk)^T)*scale + causal) @ (x@wv) @ wo

Sharding: 8 cores = DP(2 batches) x TP(4 head-groups).  Core c = 4*b + g
handles batch b, q-heads [8g..8g+8), kv-heads [2g..2g+2).  Each core
produces a partial [S, D] o-proj contribution; the host sums the 4
partials per batch (the "all-reduce" of row-parallel wo).

Software-pipelined loop over the 4 sequence blocks sb:
  P(sb): QKV projection of s-columns [512*sb, 512*sb+512) in bf16.
         Q^T/K^T produced in [hd, s] layout with RoPE via a signed
         permutation matmul (bf16); V produced in [hd, s] then
         PE-transposed to natural [s, hd].
  A(sb) ∥ O(sb-1): causal attention for q-block sb over keys
         [0, 512*sb+512).  S^T tiles = K^T.T @ Q^T, P^T = exp(S^T*scale)
         on ACT (exp pairs two key tiles; diagonal pairs exp from the
         wider tile's first live column - dead columns are never read);
         the exact-diagonal 128-chunk is masked on GPSIMD.  The softmax
         denominator partial-sums on the DVE in bf16 and folds across
         partitions with one ones-matmul per head.  Because the exp
         stream makes A ACT-bound, the PREVIOUS block's o-proj tiles are
         interleaved as PE filler (4 output tiles per head; one of them
         placed to hide the 1/l reciprocal latency).  attnT is
         double-buffered; wo stays SBUF-resident all run.
  Output partials leave as fp16 (halved out-DMA); host sums the 4 TP
  partials per batch in f32.
All matmul operands are bf16/fp16 (f32 PSUM accumulation); the kernel
is tensor-engine bound at ~90% PE occupancy.
"""

import numpy as np
import ml_dtypes

import concourse.bass as bass
import concourse.tile as tile
from concourse import bacc, mybir
from concourse.bass_utils import run_bass_kernel_spmd

BF16 = mybir.dt.bfloat16
F16 = mybir.dt.float16
F32 = mybir.dt.float32
F32R = mybir.dt.float32r

B, S, D, H, KVH, HD = 2, 2048, 4096, 32, 8, 128
G = 4                      # TP groups
HPG = H // G               # q heads per core = 8
KVPG = KVH // G            # kv heads per core = 2
NW = HPG + 2 * KVPG        # 12 projection "heads" per core (k0,k1,v0,v1,q0-7)
SCALE = 1.0 / float(np.sqrt(HD))
SB = 512                   # s-block (proj free dim, q-block, unit of pipeline)
NSB = S // SB              # 4
DT = D // 128              # 32 contraction tiles
NKT = S // 128             # 16 key tiles
N_CORES = 8

_CACHE: dict = {}


def _build():
    nc = bacc.Bacc("TRN2", target_bir_lowering=False, debug=False,
                   num_devices=N_CORES)

    # ---- DRAM I/O ----
    x_t = nc.dram_tensor("x_t", [NSB, 128, DT, SB], BF16,
                         kind="ExternalInput").ap()
    w_t = nc.dram_tensor("w_t", [NW, 128, DT, 128], BF16,
                         kind="ExternalInput").ap()
    wo_t = nc.dram_tensor("wo_t", [8, 128, HPG, 512], BF16,
                          kind="ExternalInput").ap()
    cosT = nc.dram_tensor("cosT", [NSB, 128, SB], F16, kind="ExternalInput").ap()
    sinT = nc.dram_tensor("sinT", [NSB, 128, SB], F16, kind="ExternalInput").ap()
    permT = nc.dram_tensor("permT", [128, 128], BF16, kind="ExternalInput").ap()
    maskT = nc.dram_tensor("maskT", [128, 128], BF16, kind="ExternalInput").ap()
    ones_col = nc.dram_tensor("ones_col", [128, 1], BF16, kind="ExternalInput").ap()
    ones_row = nc.dram_tensor("ones_row", [1, 128], F16, kind="ExternalInput").ap()
    ident = nc.dram_tensor("ident", [128, 128], BF16, kind="ExternalInput").ap()
    out = nc.dram_tensor("out", [NSB, 8, 4, 128, 512], F16,
                         kind="ExternalOutput").ap()

    with tile.TileContext(nc) as tc:
        with (
            tc.tile_pool(name="pers", bufs=1) as pers,
            tc.tile_pool(name="work", bufs=1) as wk,
            tc.tile_pool(name="psum", bufs=1, space="PSUM") as psum,
        ):
            # long-lived SBUF tensors
            kt_sb = pers.tile([128, KVPG, S], BF16, tag="kt")      # K^T roped
            v_sb = pers.tile([128, NKT, KVPG * 128], BF16, tag="v")  # V natural
            perm_sb = pers.tile([128, 128], BF16, tag="perm")
            mask_sb = pers.tile([128, 128], BF16, tag="mask")
            onec_sb = pers.tile([128, 1], BF16, tag="onec")
            oner_sb = pers.tile([1, 128], F16, tag="oner")
            id_sb = pers.tile([128, 128], BF16, tag="ident")
            wo_all = pers.tile([128, 8, HPG, 512], BF16, tag="wo")  # resident wo
            nc.gpsimd.dma_start(out=id_sb, in_=ident)
            nc.gpsimd.dma_start(out=perm_sb, in_=permT)
            nc.gpsimd.dma_start(out=mask_sb, in_=maskT)
            nc.gpsimd.dma_start(out=onec_sb, in_=ones_col)
            nc.gpsimd.dma_start(out=oner_sb, in_=ones_row)

            def load_rope(sb):
                cb = wk.tile([128, SB], F16, tag="cosb", bufs=2, name="cosb")
                sb_ = wk.tile([128, SB], F16, tag="sinb", bufs=2, name="sinb")
                nc.gpsimd.dma_start(out=cb, in_=cosT[sb])
                nc.gpsimd.dma_start(out=sb_, in_=sinT[sb])
                return cb, sb_

            rope_tbl = load_rope(0)

            def load_xp(sb, chunks=(slice(0, 16), slice(16, 32))):
                xp = wk.tile([128, DT, SB], BF16, tag="xp", bufs=1, name="xp")
                for c in chunks:
                    nc.gpsimd.dma_start(out=xp[:, c, :], in_=x_t[sb, :, c, :])
                return xp

            def load_wh(w_idx, split=1):
                wh = wk.tile([128, DT, 128], BF16, tag="wh", bufs=2, name="wh")
                n = DT // split
                for hc in range(split):
                    nc.sync.dma_start(
                        out=wh[:, hc * n:(hc + 1) * n, :],
                        in_=w_t[w_idx, :, hc * n:(hc + 1) * n, :])
                return wh

            # startup: sync is dedicated to the x panel (the first proj
            # chains sweep all of it), wh0 on scalar, wh1 on gpsimd; the
            # first two wo blocks queue behind everything critical
            wh0 = wk.tile([128, DT, 128], BF16, tag="wh", bufs=2, name="wh")
            wh1 = wk.tile([128, DT, 128], BF16, tag="wh", bufs=2, name="wh")
            xp = wk.tile([128, DT, SB], BF16, tag="xp", bufs=1, name="xp")
            nc.sync.dma_start(out=xp[:, 0:4, :], in_=x_t[0, :, 0:4, :])
            nc.scalar.dma_start(out=wh0[:, 0:4, :], in_=w_t[0, :, 0:4, :])
            nc.scalar.dma_start(out=wh1[:, 0:4, :], in_=w_t[1, :, 0:4, :])
            nc.sync.dma_start(out=xp[:, 4:8, :], in_=x_t[0, :, 4:8, :])
            nc.scalar.dma_start(out=wh0[:, 4:16, :], in_=w_t[0, :, 4:16, :])
            nc.scalar.dma_start(out=wh1[:, 4:16, :], in_=w_t[1, :, 4:16, :])
            nc.sync.dma_start(out=xp[:, 8:16, :], in_=x_t[0, :, 8:16, :])
            nc.scalar.dma_start(out=wh0[:, 16:32, :], in_=w_t[0, :, 16:32, :])
            nc.scalar.dma_start(out=wh1[:, 16:32, :], in_=w_t[1, :, 16:32, :])
            nc.sync.dma_start(out=xp[:, 16:24, :], in_=x_t[0, :, 16:24, :])
            nc.sync.dma_start(out=xp[:, 24:32, :], in_=x_t[0, :, 24:32, :])
            for dblk in range(2):
                nc.gpsimd.dma_start(out=wo_all[:, dblk], in_=wo_t[dblk])
            pending_wh = {0: wh0, 1: wh1}
            o_work = []
            for sb in range(NSB):
                scols = slice(sb * SB, (sb + 1) * SB)
                cos_blk, sin_blk = rope_tbl
                if sb + 1 < NSB:
                    rope_tbl = load_rope(sb + 1)

                # ============ P(sb): QKV projection + RoPE ============
                rope_pending = []

                def flush_rope_one():
                    raw, dst = rope_pending.pop(0)
                    pp = psum.tile([128, SB], F32, tag="mm", bufs=2,
                                   padded_shape=[128, SB * 2], name="pp")
                    nc.tensor.matmul(pp, perm_sb, raw, start=True, stop=True)
                    nc.vector.tensor_mul(dst, pp, sin_blk)
                    nc.vector.tensor_add(dst, dst, t1s.pop(id(raw)))

                def flush_rope():
                    while rope_pending:
                        flush_rope_one()

                t1s = {}

                def rope_head(acc, w_idx):
                    # PSUM -> raw/t1, queue the rope flush for this head
                    raw = wk.tile([128, SB], BF16, tag="raw", bufs=2)
                    nc.vector.tensor_copy(raw, acc)
                    t1 = wk.tile([128, SB], F32, tag="t1", bufs=2)
                    nc.vector.tensor_mul(t1, raw, cos_blk)
                    t1s[id(raw)] = t1
                    if w_idx < 2:
                        dst = kt_sb[:, w_idx, scols]
                    else:
                        dst = qt[:, w_idx - 4, :]
                    rope_pending.append((raw, dst))
                    if len(rope_pending) > 1:
                        flush_rope_one()

                if sb == 0:
                    # the first chain is DMA-throttled (one head sweeps the
                    # x panel at ~2x fabric rate); interleaving the two
                    # k-heads halves the per-chunk demand rate
                    whA = pending_wh.pop(0)
                    whB = pending_wh.pop(1)
                    accA = psum.tile([128, SB], F32, tag="acc", bufs=2,
                                     name="acc")
                    accB = psum.tile([128, SB], F32, tag="acc", bufs=2,
                                     name="acc")
                    for dt_i in range(DT):
                        nc.tensor.matmul(
                            accA, whA[:, dt_i, :], xp[:, dt_i, :],
                            start=(dt_i == 0), stop=(dt_i == DT - 1))
                        nc.tensor.matmul(
                            accB, whB[:, dt_i, :], xp[:, dt_i, :],
                            start=(dt_i == 0), stop=(dt_i == DT - 1))
                    rope_head(accA, 0)
                    rope_head(accB, 1)
                    head_list = range(2, NW)
                else:
                    head_list = range(NW)

                for w_idx in head_list:
                    wh = pending_wh.pop(w_idx, None)
                    if wh is None:
                        wh = load_wh(w_idx)
                    if 2 <= w_idx < 4:
                        # v head: [hd, s] proj then PE-transpose to natural
                        kvs = w_idx - 2
                        acc = psum.tile([128, SB], F32, tag="acc", bufs=2,
                                        name="acc")
                        for dt_i in range(DT):
                            nc.tensor.matmul(
                                acc, wh[:, dt_i, :], xp[:, dt_i, :],
                                start=(dt_i == 0), stop=(dt_i == DT - 1))
                        vtp = wk.tile([128, SB], BF16, tag="vtp", bufs=1)
                        nc.vector.tensor_copy(vtp, acc)
                        for blk in range(SB // 128):
                            tp = psum.tile([128, 128], BF16, tag="mm", bufs=2,
                                           padded_shape=[128, SB * 4], name="tp")
                            nc.tensor.transpose(
                                tp, vtp[:, blk * 128:(blk + 1) * 128], id_sb)
                            nc.vector.tensor_copy(
                                v_sb[:, sb * 4 + blk,
                                     kvs * 128:(kvs + 1) * 128], tp)
                    else:
                        # q or k head: [hd, s] layout + rope
                        acc = psum.tile([128, SB], F32, tag="acc", bufs=2)
                        for dt_i in range(DT):
                            nc.tensor.matmul(
                                acc, wh[:, dt_i, :], xp[:, dt_i, :],
                                start=(dt_i == 0), stop=(dt_i == DT - 1))
                        if w_idx == 4:
                            qt = wk.tile([128, HPG, SB], BF16, tag="qt",
                                         bufs=1)
                        rope_head(acc, w_idx)

                flush_rope()

                if sb == 0:
                    # remaining wo blocks, queued after P(0)'s weight stream
                    for dblk in range(2, 8):
                        nc.sync.dma_start(out=wo_all[:, dblk], in_=wo_t[dblk])

                # prefetch next s-block activations; the DMA starts as soon
                # as P(sb)'s last read of the single xp buffer retires and
                # hides under A(sb)+O(sb)
                if sb + 1 < NSB:
                    xp = load_xp(sb + 1)

                # ============ A(sb): attention q-block qi=sb ============
                # Scores/exp for head h+1 are interleaved (cross-head
                # software pipeline) with PV/l-row of head h so the tensor
                # engine never idles waiting on the ACT exp stream (idle
                # gaps re-engage the HAM clock throttle).
                nkt = 4 * sb + 4

                def q0(kti):  # first causally-live query column for key tile
                    return 128 * (kti - 4 * sb) if kti >= 4 * sb else 0

                # every task is a pair of key tiles sharing one wide exp;
                # diagonal pairs exp from the wider tile's first live column
                # (the partner's dead columns are never read downstream)
                tasks = []
                for h in range(HPG):
                    tasks += [(h, (k, k + 1)) for k in range(0, nkt, 2)]
                pt_tiles = {}
                state = {"ti": 0, "tiles": 0}

                def issue_task():
                    if state["ti"] >= len(tasks):
                        return
                    h2, ks = tasks[state["ti"]]
                    state["ti"] += 1
                    state["tiles"] += 2
                    if ks[0] == 0:
                        pt_tiles[h2] = wk.tile([128, NKT, SB], BF16, tag="pt",
                                               bufs=2, name="pt")
                    pt = pt_tiles[h2]
                    kvs2 = h2 // (HPG // KVPG)
                    st = psum.tile([128, 2, SB], F32, tag="mm", bufs=2,
                                   name="st")
                    for j, k in enumerate(ks):
                        qo = q0(k)
                        nc.tensor.matmul(
                            st[:, j, qo:],
                            kt_sb[:, kvs2, k * 128:(k + 1) * 128],
                            qt[:, h2, qo:],
                            start=True, stop=True)
                    eqo = q0(ks[0])
                    nc.scalar.activation(
                        pt[:, ks[0]:ks[0] + 2, eqo:], st[:, :, eqo:],
                        mybir.ActivationFunctionType.Exp, scale=SCALE)
                    for k in ks:
                        if k >= 4 * sb:  # diagonal tile: mask its 128-chunk
                            qo = q0(k)
                            nc.gpsimd.tensor_mul(
                                pt[:, k, qo:qo + 128],
                                pt[:, k, qo:qo + 128], mask_sb)

                def emit_o_tile(at_prev, psb, dblk, qs):
                    # one o-proj output tile of s-block psb (PE filler work)
                    ops = psum.tile([128, 512], F32, tag="alt", bufs=2,
                                    name="ops")
                    for hp in range(HPG):
                        nc.tensor.matmul(
                            ops,
                            at_prev[:, hp, qs * 128:(qs + 1) * 128],
                            wo_all[:, dblk, hp, :],
                            start=(hp == 0), stop=(hp == HPG - 1))
                    o_sb = wk.tile([128, 512], F16, tag="osb", bufs=4)
                    nc.vector.tensor_copy(o_sb, ops)
                    nc.scalar.dma_start(out=out[psb, dblk, qs], in_=o_sb)

                while state["tiles"] < 4 and state["ti"] < len(tasks):
                    issue_task()
                attnT = wk.tile([128, HPG, SB], BF16, tag="attnT", bufs=2)
                consumed = 0
                for h in range(HPG):
                    kvs = h // (HPG // KVPG)
                    pt = pt_tiles[h]
                    oT = psum.tile([128, SB], F32, tag="acc", bufs=2)
                    lrow = psum.tile([1, SB], F32, tag="alt", bufs=2,
                                     name="lrow")
                    # softmax denominator: partial-sum all key tiles on the
                    # DVE in bf16 (2x rate), fold across partitions with one
                    # ones-matmul per head
                    lp = wk.tile([128, SB], BF16, tag="lp", bufs=1)
                    for kti in range(nkt):
                        qo = q0(kti)
                        nc.tensor.matmul(
                            oT[:, qo:],
                            v_sb[:, kti, kvs * 128:(kvs + 1) * 128],
                            pt[:, kti, qo:],
                            start=(kti == 0), stop=(kti == nkt - 1))
                        if kti == 0:
                            nc.vector.tensor_copy(lp, pt[:, 0, :])
                        else:
                            nc.vector.tensor_add(
                                lp[:, qo:], lp[:, qo:], pt[:, kti, qo:])
                        consumed += 1
                        while (state["tiles"] < consumed + 4
                               and state["ti"] < len(tasks)):
                            issue_task()
                    nc.tensor.matmul(lrow, onec_sb, lp,
                                     start=True, stop=True)
                    # finalize: attnT = oT * bcast(1/l).  One o-proj filler
                    # tile of the previous s-block hides the DVE reciprocal
                    # latency between the fold and bc matmuls; three more
                    # keep the PE fed while the ACT exp stream catches up.
                    # (recip is issued BEFORE the filler so the filler's DVE
                    # drain doesn't delay it on the in-order DVE queue)
                    linv = wk.tile([1, SB], F32, tag="linv", bufs=1)
                    nc.vector.reciprocal_approx_fast(linv, lrow)
                    linv_r = wk.tile([1, SB], F16, tag="linvr", bufs=1)
                    nc.vector.tensor_copy(linv_r, linv)
                    if o_work:
                        emit_o_tile(*o_work.pop(0))
                    bc = psum.tile([128, SB], F32, tag="alt", bufs=2,
                                   name="bc")
                    nc.tensor.matmul(bc, oner_sb, linv_r,
                                     start=True, stop=True)
                    bc_sb = wk.tile([128, SB], F16, tag="bcsb", bufs=1)
                    nc.vector.tensor_copy(bc_sb, bc)
                    nc.vector.tensor_mul(attnT[:, h, :], oT, bc_sb)
                    for _ in range(3):
                        if o_work:
                            emit_o_tile(*o_work.pop(0))

                # o-proj of THIS s-block: deferred into A(sb+1) as PE filler
                # (the ACT-bound exp stream there leaves PE slack); the last
                # block's runs right here
                while o_work:
                    emit_o_tile(*o_work.pop(0))
                o_work = [(attnT, sb, dblk, qs)
                          for dblk in range(8) for qs in range(4)]
                if sb + 1 == NSB:
                    while o_work:
                        emit_o_tile(*o_work.pop(0))

                # prefetch next s-block weights ahead of the out-DMA backlog
                if sb + 1 < NSB:
                    pending_wh = {0: load_wh(0), 1: load_wh(1)}
    nc.compile()
    return nc


def _host_inputs(x, wq, wk, wv, wo, cos, sin):
    """Build the 8 per-core input maps (all host-side prep)."""
    x = np.asarray(x, np.float32)
    wq = np.asarray(wq, np.float32)
    wk = np.asarray(wk, np.float32)
    wv = np.asarray(wv, np.float32)
    wo = np.asarray(wo, np.float32)
    cos = np.asarray(cos, np.float32)
    sin = np.asarray(sin, np.float32)

    # [NSB, 128, SB] fp16 per-block rotary tables (rows repeated in pairs)
    cosT = np.ascontiguousarray(
        np.repeat(cos.T, 2, axis=0).reshape(128, NSB, SB).transpose(1, 0, 2)
    ).astype(np.float16)
    sinT = np.ascontiguousarray(
        np.repeat(sin.T, 2, axis=0).reshape(128, NSB, SB).transpose(1, 0, 2)
    ).astype(np.float16)
    permT = np.zeros((128, 128), ml_dtypes.bfloat16)
    idx = np.arange(64)
    permT[2 * idx + 1, 2 * idx] = -1.0
    permT[2 * idx, 2 * idx + 1] = 1.0
    kk = np.arange(128)[:, None]
    tt = np.arange(128)[None, :]
    maskT = (tt >= kk).astype(ml_dtypes.bfloat16)      # [128, 128]
    ones_col = np.ones((128, 1), ml_dtypes.bfloat16)
    ones_row = np.ones((1, 128), np.float16)
    ident = np.eye(128, dtype=ml_dtypes.bfloat16)

    def tile_w(w_col):  # [D, 128] -> [128, DT, 128]
        return w_col.reshape(DT, 128, 128).transpose(1, 0, 2)

    x_ts = [np.ascontiguousarray(
        x[b].T.reshape(DT, 128, NSB, SB).transpose(2, 1, 0, 3)
    ).astype(ml_dtypes.bfloat16) for b in range(B)]
    in_maps = []
    for core in range(N_CORES):
        b, g = divmod(core, G)
        w_np = np.empty((NW, 128, DT, 128), np.float32)
        for j in range(KVPG):
            w_np[j] = tile_w(wk[:, (g * KVPG + j) * 128:(g * KVPG + j + 1) * 128])
        for j in range(KVPG):
            w_np[KVPG + j] = tile_w(
                wv[:, (g * KVPG + j) * 128:(g * KVPG + j + 1) * 128])
        for j in range(HPG):
            w_np[2 * KVPG + j] = tile_w(
                wq[:, (g * HPG + j) * 128:(g * HPG + j + 1) * 128])
        wo_g = wo[g * HPG * HD:(g + 1) * HPG * HD, :]          # [1024, D]
        wo_np = np.ascontiguousarray(
            wo_g.reshape(HPG, 128, 8, 512).transpose(2, 1, 0, 3)
        ).astype(ml_dtypes.bfloat16)                           # [8, 128, HPG, 512]
        in_maps.append({
            "x_t": x_ts[b], "w_t": w_np.astype(ml_dtypes.bfloat16),
            "wo_t": wo_np,
            "cosT": cosT, "sinT": sinT, "permT": permT, "maskT": maskT,
            "ones_col": ones_col, "ones_row": ones_row, "ident": ident,
        })
    return in_maps


def kernel(x, wq, wk, wv, wo, cos, sin, mask, start_pos):
    assert int(start_pos) == 0, "kernel compiled for prefill (start_pos=0)"
    if "nc" not in _CACHE:
        _CACHE["nc"] = _build()
    nc = _CACHE["nc"]
    in_maps = _host_inputs(x, wq, wk, wv, wo, cos, sin)
    res = run_bass_kernel_spmd(nc, in_maps, list(range(N_CORES)))

    def unpack(o):  # [NSB, 8, 4, 128, 512] -> [S, D]
        return np.ascontiguousarray(
            np.transpose(o, (0, 2, 3, 1, 4)).reshape(S, D))

    full = np.empty((B, S, D), np.float32)
    for b in range(B):
        acc = res.results[4 * b]["out"].astype(np.float32)
        for g in range(1, G):
            acc = acc + res.results[4 * b + g]["out"]
        full[b] = unpack(acc)
    return full



# revision 36
# speedup vs baseline: 1.0252x; 1.0146x over previous
"""Trainium2 Bass kernel for GQA attention layer (Llama-style, prefill).

Full computation:  out = softmax((rope(x@wq) @ rope(x@wk)^T)*scale + causal) @ (x@wv) @ wo

Sharding: 8 cores = DP(2 batches) x TP(4 head-groups).  Core c = 4*b + g
handles batch b, q-heads [8g..8g+8), kv-heads [2g..2g+2).  Each core
produces a partial [S, D] o-proj contribution; the host sums the 4
partials per batch (the "all-reduce" of row-parallel wo).

Software-pipelined loop over the 4 sequence blocks sb:
  P(sb): QKV projection of s-columns [512*sb, 512*sb+512) in bf16.
         Q^T/K^T produced in [hd, s] layout with RoPE via a signed
         permutation matmul (bf16); V produced in [hd, s] then
         PE-transposed to natural [s, hd].
  A(sb) ∥ O(sb-1): causal attention for q-block sb over keys
         [0, 512*sb+512).  S^T tiles = K^T.T @ Q^T, P^T = exp(S^T*scale)
         on ACT (exp pairs two key tiles; diagonal pairs exp from the
         wider tile's first live column - dead columns are never read);
         the exact-diagonal 128-chunk is masked on GPSIMD.  The softmax
         denominator partial-sums on the DVE in bf16 and folds across
         partitions with one ones-matmul per head.  Because the exp
         stream makes A ACT-bound, the PREVIOUS block's o-proj tiles are
         interleaved as PE filler (4 output tiles per head; one of them
         placed to hide the 1/l reciprocal latency).  attnT is
         double-buffered; wo stays SBUF-resident all run.
  Output partials leave as fp16 (halved out-DMA); host sums the 4 TP
  partials per batch in f32.
All matmul operands are bf16/fp16 (f32 PSUM accumulation); the kernel
is tensor-engine bound at ~90% PE occupancy.
"""

import numpy as np
import ml_dtypes

import concourse.bass as bass
import concourse.tile as tile
from concourse import bacc, mybir
from concourse.bass_utils import run_bass_kernel_spmd

BF16 = mybir.dt.bfloat16
F16 = mybir.dt.float16
F32 = mybir.dt.float32
F32R = mybir.dt.float32r

B, S, D, H, KVH, HD = 2, 2048, 4096, 32, 8, 128
G = 4                      # TP groups
HPG = H // G               # q heads per core = 8
KVPG = KVH // G            # kv heads per core = 2
NW = HPG + 2 * KVPG        # 12 projection "heads" per core (k0,k1,v0,v1,q0-7)
SCALE = 1.0 / float(np.sqrt(HD))
SB = 512                   # s-block (proj free dim, q-block, unit of pipeline)
NSB = S // SB              # 4
DT = D // 128              # 32 contraction tiles
NKT = S // 128             # 16 key tiles
N_CORES = 8

_CACHE: dict = {}


def _build():
    nc = bacc.Bacc("TRN2", target_bir_lowering=False, debug=False,
                   num_devices=N_CORES)

    # ---- DRAM I/O ----
    x_t = nc.dram_tensor("x_t", [NSB, 128, DT, SB], BF16,
                         kind="ExternalInput").ap()
    w_t = nc.dram_tensor("w_t", [NW, 128, DT, 128], BF16,
                         kind="ExternalInput").ap()
    wo_t = nc.dram_tensor("wo_t", [8, 128, HPG, 512], BF16,
                          kind="ExternalInput").ap()
    cosT = nc.dram_tensor("cosT", [NSB, 128, SB], F16, kind="ExternalInput").ap()
    sinT = nc.dram_tensor("sinT", [NSB, 128, SB], F16, kind="ExternalInput").ap()
    permT = nc.dram_tensor("permT", [128, 128], BF16, kind="ExternalInput").ap()
    maskT = nc.dram_tensor("maskT", [128, 128], BF16, kind="ExternalInput").ap()
    ones_col = nc.dram_tensor("ones_col", [128, 1], BF16, kind="ExternalInput").ap()
    ones_row = nc.dram_tensor("ones_row", [1, 128], F16, kind="ExternalInput").ap()
    ident = nc.dram_tensor("ident", [128, 128], BF16, kind="ExternalInput").ap()
    out = nc.dram_tensor("out", [NSB, 8, 4, 128, 512], F16,
                         kind="ExternalOutput").ap()

    with tile.TileContext(nc) as tc:
        with (
            tc.tile_pool(name="pers", bufs=1) as pers,
            tc.tile_pool(name="work", bufs=1) as wk,
            tc.tile_pool(name="psum", bufs=1, space="PSUM") as psum,
        ):
            # long-lived SBUF tensors
            kt_sb = pers.tile([128, KVPG, S], BF16, tag="kt")      # K^T roped
            v_sb = pers.tile([128, NKT, KVPG * 128], BF16, tag="v")  # V natural
            perm_sb = pers.tile([128, 128], BF16, tag="perm")
            mask_sb = pers.tile([128, 128], BF16, tag="mask")
            onec_sb = pers.tile([128, 1], BF16, tag="onec")
            oner_sb = pers.tile([1, 128], F16, tag="oner")
            id_sb = pers.tile([128, 128], BF16, tag="ident")
            wo_all = pers.tile([128, 8, HPG, 512], BF16, tag="wo")  # resident wo
            nc.gpsimd.dma_start(out=id_sb, in_=ident)
            nc.gpsimd.dma_start(out=perm_sb, in_=permT)
            nc.gpsimd.dma_start(out=mask_sb, in_=maskT)
            nc.gpsimd.dma_start(out=onec_sb, in_=ones_col)
            nc.gpsimd.dma_start(out=oner_sb, in_=ones_row)

            def load_rope(sb):
                cb = wk.tile([128, SB], F16, tag="cosb", bufs=2, name="cosb")
                sb_ = wk.tile([128, SB], F16, tag="sinb", bufs=2, name="sinb")
                nc.gpsimd.dma_start(out=cb, in_=cosT[sb])
                nc.gpsimd.dma_start(out=sb_, in_=sinT[sb])
                return cb, sb_

            rope_tbl = load_rope(0)

            def load_xp(sb, chunks=(slice(0, 16), slice(16, 32))):
                xp = wk.tile([128, DT, SB], BF16, tag="xp", bufs=1, name="xp")
                for c in chunks:
                    nc.gpsimd.dma_start(out=xp[:, c, :], in_=x_t[sb, :, c, :])
                return xp

            def load_wh(w_idx, split=1):
                wh = wk.tile([128, DT, 128], BF16, tag="wh", bufs=2, name="wh")
                n = DT // split
                for hc in range(split):
                    nc.sync.dma_start(
                        out=wh[:, hc * n:(hc + 1) * n, :],
                        in_=w_t[w_idx, :, hc * n:(hc + 1) * n, :])
                return wh

            # startup: sync is dedicated to the x panel (the first proj
            # chains sweep all of it), wh0 on scalar, wh1 on gpsimd; the
            # first two wo blocks queue behind everything critical
            wh0 = wk.tile([128, DT, 128], BF16, tag="wh", bufs=2, name="wh")
            wh1 = wk.tile([128, DT, 128], BF16, tag="wh", bufs=2, name="wh")
            xp = wk.tile([128, DT, SB], BF16, tag="xp", bufs=1, name="xp")
            for dblk in range(2):
                nc.gpsimd.dma_start(out=wo_all[:, dblk], in_=wo_t[dblk])
            for c in range(8):
                cs = slice(c * 4, (c + 1) * 4)
                nc.sync.dma_start(out=xp[:, cs, :], in_=x_t[0, :, cs, :])
                nc.scalar.dma_start(out=wh0[:, cs, :], in_=w_t[0, :, cs, :])
                nc.scalar.dma_start(out=wh1[:, cs, :], in_=w_t[1, :, cs, :])
            pending_wh = {0: wh0, 1: wh1}
            o_work = []
            for sb in range(NSB):
                scols = slice(sb * SB, (sb + 1) * SB)
                cos_blk, sin_blk = rope_tbl
                if sb + 1 < NSB:
                    rope_tbl = load_rope(sb + 1)

                # ============ P(sb): QKV projection + RoPE ============
                rope_pending = []

                def flush_rope_one():
                    raw, dst = rope_pending.pop(0)
                    pp = psum.tile([128, SB], F32, tag="mm", bufs=2,
                                   padded_shape=[128, SB * 2], name="pp")
                    nc.tensor.matmul(pp, perm_sb, raw, start=True, stop=True)
                    nc.vector.tensor_mul(dst, pp, sin_blk)
                    nc.vector.tensor_add(dst, dst, t1s.pop(id(raw)))

                def flush_rope():
                    while rope_pending:
                        flush_rope_one()

                t1s = {}

                def rope_head(acc, w_idx):
                    # PSUM -> raw/t1, queue the rope flush for this head
                    raw = wk.tile([128, SB], BF16, tag="raw", bufs=2)
                    nc.vector.tensor_copy(raw, acc)
                    t1 = wk.tile([128, SB], F32, tag="t1", bufs=2)
                    nc.vector.tensor_mul(t1, raw, cos_blk)
                    t1s[id(raw)] = t1
                    if w_idx < 2:
                        dst = kt_sb[:, w_idx, scols]
                    else:
                        dst = qt[:, w_idx - 4, :]
                    rope_pending.append((raw, dst))
                    if len(rope_pending) > 1:
                        flush_rope_one()

                if sb == 0:
                    # the first chain is DMA-throttled (one head sweeps the
                    # x panel at ~2x fabric rate); interleaving the two
                    # k-heads halves the per-chunk demand rate
                    whA = pending_wh.pop(0)
                    whB = pending_wh.pop(1)
                    accA = psum.tile([128, SB], F32, tag="acc", bufs=2,
                                     name="acc")
                    accB = psum.tile([128, SB], F32, tag="acc", bufs=2,
                                     name="acc")
                    for dt_i in range(DT):
                        nc.tensor.matmul(
                            accA, whA[:, dt_i, :], xp[:, dt_i, :],
                            start=(dt_i == 0), stop=(dt_i == DT - 1))
                        nc.tensor.matmul(
                            accB, whB[:, dt_i, :], xp[:, dt_i, :],
                            start=(dt_i == 0), stop=(dt_i == DT - 1))
                    rope_head(accA, 0)
                    rope_head(accB, 1)
                    head_list = range(2, NW)
                else:
                    head_list = range(NW)

                for w_idx in head_list:
                    wh = pending_wh.pop(w_idx, None)
                    if wh is None:
                        wh = load_wh(w_idx)
                    if 2 <= w_idx < 4:
                        # v head: [hd, s] proj then PE-transpose to natural
                        kvs = w_idx - 2
                        acc = psum.tile([128, SB], F32, tag="acc", bufs=2,
                                        name="acc")
                        for dt_i in range(DT):
                            nc.tensor.matmul(
                                acc, wh[:, dt_i, :], xp[:, dt_i, :],
                                start=(dt_i == 0), stop=(dt_i == DT - 1))
                        vtp = wk.tile([128, SB], BF16, tag="vtp", bufs=1)
                        nc.vector.tensor_copy(vtp, acc)
                        for blk in range(SB // 128):
                            tp = psum.tile([128, 128], BF16, tag="mm", bufs=2,
                                           padded_shape=[128, SB * 4], name="tp")
                            nc.tensor.transpose(
                                tp, vtp[:, blk * 128:(blk + 1) * 128], id_sb)
                            nc.vector.tensor_copy(
                                v_sb[:, sb * 4 + blk,
                                     kvs * 128:(kvs + 1) * 128], tp)
                    else:
                        # q or k head: [hd, s] layout + rope
                        acc = psum.tile([128, SB], F32, tag="acc", bufs=2)
                        for dt_i in range(DT):
                            nc.tensor.matmul(
                                acc, wh[:, dt_i, :], xp[:, dt_i, :],
                                start=(dt_i == 0), stop=(dt_i == DT - 1))
                        if w_idx == 4:
                            qt = wk.tile([128, HPG, SB], BF16, tag="qt",
                                         bufs=1)
                        rope_head(acc, w_idx)

                flush_rope()

                if sb == 0:
                    # remaining wo blocks, queued after P(0)'s weight stream
                    for dblk in range(2, 8):
                        nc.sync.dma_start(out=wo_all[:, dblk], in_=wo_t[dblk])

                # prefetch next s-block activations; the DMA starts as soon
                # as P(sb)'s last read of the single xp buffer retires and
                # hides under A(sb)+O(sb)
                if sb + 1 < NSB:
                    xp = load_xp(sb + 1)

                # ============ A(sb): attention q-block qi=sb ============
                # Scores/exp for head h+1 are interleaved (cross-head
                # software pipeline) with PV/l-row of head h so the tensor
                # engine never idles waiting on the ACT exp stream (idle
                # gaps re-engage the HAM clock throttle).
                nkt = 4 * sb + 4

                def q0(kti):  # first causally-live query column for key tile
                    return 128 * (kti - 4 * sb) if kti >= 4 * sb else 0

                # every task is a pair of key tiles sharing one wide exp;
                # diagonal pairs exp from the wider tile's first live column
                # (the partner's dead columns are never read downstream)
                tasks = []
                for h in range(HPG):
                    tasks += [(h, (k, k + 1)) for k in range(0, nkt, 2)]
                pt_tiles = {}
                state = {"ti": 0, "tiles": 0}

                def issue_task():
                    if state["ti"] >= len(tasks):
                        return
                    h2, ks = tasks[state["ti"]]
                    state["ti"] += 1
                    state["tiles"] += 2
                    if ks[0] == 0:
                        pt_tiles[h2] = wk.tile([128, NKT, SB], BF16, tag="pt",
                                               bufs=2, name="pt")
                    pt = pt_tiles[h2]
                    kvs2 = h2 // (HPG // KVPG)
                    st = psum.tile([128, 2, SB], F32, tag="mm", bufs=2,
                                   name="st")
                    for j, k in enumerate(ks):
                        qo = q0(k)
                        nc.tensor.matmul(
                            st[:, j, qo:],
                            kt_sb[:, kvs2, k * 128:(k + 1) * 128],
                            qt[:, h2, qo:],
                            start=True, stop=True)
                    eqo = q0(ks[0])
                    nc.scalar.activation(
                        pt[:, ks[0]:ks[0] + 2, eqo:], st[:, :, eqo:],
                        mybir.ActivationFunctionType.Exp, scale=SCALE)
                    for k in ks:
                        if k >= 4 * sb:  # diagonal tile: mask its 128-chunk
                            qo = q0(k)
                            nc.gpsimd.tensor_mul(
                                pt[:, k, qo:qo + 128],
                                pt[:, k, qo:qo + 128], mask_sb)

                def emit_o_tile(at_prev, psb, dblk, qs):
                    # one o-proj output tile of s-block psb (PE filler work)
                    ops = psum.tile([128, 512], F32, tag="alt", bufs=2,
                                    name="ops")
                    for hp in range(HPG):
                        nc.tensor.matmul(
                            ops,
                            at_prev[:, hp, qs * 128:(qs + 1) * 128],
                            wo_all[:, dblk, hp, :],
                            start=(hp == 0), stop=(hp == HPG - 1))
                    o_sb = wk.tile([128, 512], F16, tag="osb", bufs=4)
                    nc.vector.tensor_copy(o_sb, ops)
                    nc.scalar.dma_start(out=out[psb, dblk, qs], in_=o_sb)

                while state["tiles"] < 4 and state["ti"] < len(tasks):
                    issue_task()
                attnT = wk.tile([128, HPG, SB], BF16, tag="attnT", bufs=2)
                consumed = 0
                for h in range(HPG):
                    kvs = h // (HPG // KVPG)
                    pt = pt_tiles[h]
                    oT = psum.tile([128, SB], F32, tag="acc", bufs=2)
                    lrow = psum.tile([1, SB], F32, tag="alt", bufs=2,
                                     name="lrow")
                    # softmax denominator: partial-sum all key tiles on the
                    # DVE in bf16 (2x rate), fold across partitions with one
                    # ones-matmul per head
                    lp = wk.tile([128, SB], BF16, tag="lp", bufs=1)
                    for kti in range(nkt):
                        qo = q0(kti)
                        nc.tensor.matmul(
                            oT[:, qo:],
                            v_sb[:, kti, kvs * 128:(kvs + 1) * 128],
                            pt[:, kti, qo:],
                            start=(kti == 0), stop=(kti == nkt - 1))
                        if kti == 0:
                            nc.vector.tensor_copy(lp, pt[:, 0, :])
                        else:
                            nc.vector.tensor_add(
                                lp[:, qo:], lp[:, qo:], pt[:, kti, qo:])
                        consumed += 1
                        while (state["tiles"] < consumed + 4
                               and state["ti"] < len(tasks)):
                            issue_task()
                    nc.tensor.matmul(lrow, onec_sb, lp,
                                     start=True, stop=True)
                    # finalize: attnT = oT * bcast(1/l).  One o-proj filler
                    # tile of the previous s-block hides the DVE reciprocal
                    # latency between the fold and bc matmuls; three more
                    # keep the PE fed while the ACT exp stream catches up.
                    # (recip is issued BEFORE the filler so the filler's DVE
                    # drain doesn't delay it on the in-order DVE queue)
                    linv = wk.tile([1, SB], F32, tag="linv", bufs=1)
                    nc.vector.reciprocal_approx_fast(linv, lrow)
                    linv_r = wk.tile([1, SB], F16, tag="linvr", bufs=1)
                    nc.vector.tensor_copy(linv_r, linv)
                    if o_work:
                        emit_o_tile(*o_work.pop(0))
                    bc = psum.tile([128, SB], F32, tag="alt", bufs=2,
                                   name="bc")
                    nc.tensor.matmul(bc, oner_sb, linv_r,
                                     start=True, stop=True)
                    bc_sb = wk.tile([128, SB], F16, tag="bcsb", bufs=1)
                    nc.vector.tensor_copy(bc_sb, bc)
                    nc.vector.tensor_mul(attnT[:, h, :], oT, bc_sb)
                    for _ in range(3):
                        if o_work:
                            emit_o_tile(*o_work.pop(0))

                # o-proj of THIS s-block: deferred into A(sb+1) as PE filler
                # (the ACT-bound exp stream there leaves PE slack); the last
                # block's runs right here
                while o_work:
                    emit_o_tile(*o_work.pop(0))
                o_work = [(attnT, sb, dblk, qs)
                          for dblk in range(8) for qs in range(4)]
                if sb + 1 == NSB:
                    while o_work:
                        emit_o_tile(*o_work.pop(0))

                # prefetch next s-block weights ahead of the out-DMA backlog
                if sb + 1 < NSB:
                    pending_wh = {0: load_wh(0), 1: load_wh(1)}
    nc.compile()
    return nc


def _host_inputs(x, wq, wk, wv, wo, cos, sin):
    """Build the 8 per-core input maps (all host-side prep)."""
    x = np.asarray(x, np.float32)
    wq = np.asarray(wq, np.float32)
    wk = np.asarray(wk, np.float32)
    wv = np.asarray(wv, np.float32)
    wo = np.asarray(wo, np.float32)
    cos = np.asarray(cos, np.float32)
    sin = np.asarray(sin, np.float32)

    # [NSB, 128, SB] fp16 per-block rotary tables (rows repeated in pairs)
    cosT = np.ascontiguousarray(
        np.repeat(cos.T, 2, axis=0).reshape(128, NSB, SB).transpose(1, 0, 2)
    ).astype(np.float16)
    sinT = np.ascontiguousarray(
        np.repeat(sin.T, 2, axis=0).reshape(128, NSB, SB).transpose(1, 0, 2)
    ).astype(np.float16)
    permT = np.zeros((128, 128), ml_dtypes.bfloat16)
    idx = np.arange(64)
    permT[2 * idx + 1, 2 * idx] = -1.0
    permT[2 * idx, 2 * idx + 1] = 1.0
    kk = np.arange(128)[:, None]
    tt = np.arange(128)[None, :]
    maskT = (tt >= kk).astype(ml_dtypes.bfloat16)      # [128, 128]
    ones_col = np.ones((128, 1), ml_dtypes.bfloat16)
    ones_row = np.ones((1, 128), np.float16)
    ident = np.eye(128, dtype=ml_dtypes.bfloat16)

    def tile_w(w_col):  # [D, 128] -> [128, DT, 128]
        return w_col.reshape(DT, 128, 128).transpose(1, 0, 2)

    x_ts = [np.ascontiguousarray(
        x[b].T.reshape(DT, 128, NSB, SB).transpose(2, 1, 0, 3)
    ).astype(ml_dtypes.bfloat16) for b in range(B)]
    in_maps = []
    for core in range(N_CORES):
        b, g = divmod(core, G)
        w_np = np.empty((NW, 128, DT, 128), np.float32)
        for j in range(KVPG):
            w_np[j] = tile_w(wk[:, (g * KVPG + j) * 128:(g * KVPG + j + 1) * 128])
        for j in range(KVPG):
            w_np[KVPG + j] = tile_w(
                wv[:, (g * KVPG + j) * 128:(g * KVPG + j + 1) * 128])
        for j in range(HPG):
            w_np[2 * KVPG + j] = tile_w(
                wq[:, (g * HPG + j) * 128:(g * HPG + j + 1) * 128])
        wo_g = wo[g * HPG * HD:(g + 1) * HPG * HD, :]          # [1024, D]
        wo_np = np.ascontiguousarray(
            wo_g.reshape(HPG, 128, 8, 512).transpose(2, 1, 0, 3)
        ).astype(ml_dtypes.bfloat16)                           # [8, 128, HPG, 512]
        in_maps.append({
            "x_t": x_ts[b], "w_t": w_np.astype(ml_dtypes.bfloat16),
            "wo_t": wo_np,
            "cosT": cosT, "sinT": sinT, "permT": permT, "maskT": maskT,
            "ones_col": ones_col, "ones_row": ones_row, "ident": ident,
        })
    return in_maps


def kernel(x, wq, wk, wv, wo, cos, sin, mask, start_pos):
    assert int(start_pos) == 0, "kernel compiled for prefill (start_pos=0)"
    if "nc" not in _CACHE:
        _CACHE["nc"] = _build()
    nc = _CACHE["nc"]
    in_maps = _host_inputs(x, wq, wk, wv, wo, cos, sin)
    res = run_bass_kernel_spmd(nc, in_maps, list(range(N_CORES)))

    def unpack(o):  # [NSB, 8, 4, 128, 512] -> [S, D]
        return np.ascontiguousarray(
            np.transpose(o, (0, 2, 3, 1, 4)).reshape(S, D))

    full = np.empty((B, S, D), np.float32)
    for b in range(B):
        acc = res.results[4 * b]["out"].astype(np.float32)
        for g in range(1, G):
            acc = acc + res.results[4 * b + g]["out"]
        full[b] = unpack(acc)
    return full



# revision 38
# speedup vs baseline: 1.0374x; 1.0118x over previous
"""Trainium2 Bass kernel for GQA attention layer (Llama-style, prefill).

Full computation:  out = softmax((rope(x@wq) @ rope(x@wk)^T)*scale + causal) @ (x@wv) @ wo

Sharding: 8 cores = DP(2 batches) x TP(4 head-groups).  Core c = 4*b + g
handles batch b, q-heads [8g..8g+8), kv-heads [2g..2g+2).  Each core
produces a partial [S, D] o-proj contribution; the host sums the 4
partials per batch (the "all-reduce" of row-parallel wo).

Software-pipelined loop over the 4 sequence blocks sb:
  P(sb): QKV projection of s-columns [512*sb, 512*sb+512) in bf16.
         Q^T/K^T produced in [hd, s] layout with RoPE via a signed
         permutation matmul (bf16); V produced in [hd, s] then
         PE-transposed to natural [s, hd].
  A(sb) ∥ O(sb-1): causal attention for q-block sb over keys
         [0, 512*sb+512).  S^T tiles = K^T.T @ Q^T, P^T = exp(S^T*scale)
         on ACT (exp pairs two key tiles; diagonal pairs exp from the
         wider tile's first live column - dead columns are never read);
         the exact-diagonal 128-chunk is masked on GPSIMD.  The softmax
         denominator partial-sums on the DVE in bf16 and folds across
         partitions with one ones-matmul per head.  Because the exp
         stream makes A ACT-bound, the PREVIOUS block's o-proj tiles are
         interleaved as PE filler (4 output tiles per head; one of them
         placed to hide the 1/l reciprocal latency).  attnT is
         double-buffered; wo stays SBUF-resident all run.
  Output partials leave as fp16 (halved out-DMA); host sums the 4 TP
  partials per batch in f32.
All matmul operands are bf16/fp16 (f32 PSUM accumulation); the kernel
is tensor-engine bound at ~90% PE occupancy.
"""

import numpy as np
import ml_dtypes

import concourse.bass as bass
import concourse.tile as tile
from concourse import bacc, mybir
from concourse.bass_utils import run_bass_kernel_spmd

BF16 = mybir.dt.bfloat16
F16 = mybir.dt.float16
F32 = mybir.dt.float32
F32R = mybir.dt.float32r

B, S, D, H, KVH, HD = 2, 2048, 4096, 32, 8, 128
G = 4                      # TP groups
HPG = H // G               # q heads per core = 8
KVPG = KVH // G            # kv heads per core = 2
NW = HPG + 2 * KVPG        # 12 projection "heads" per core (k0,k1,v0,v1,q0-7)
SCALE = 1.0 / float(np.sqrt(HD))
SB = 512                   # s-block (proj free dim, q-block, unit of pipeline)
NSB = S // SB              # 4
DT = D // 128              # 32 contraction tiles
NKT = S // 128             # 16 key tiles
N_CORES = 8

_CACHE: dict = {}


def _build():
    nc = bacc.Bacc("TRN2", target_bir_lowering=False, debug=False,
                   num_devices=N_CORES)

    # ---- DRAM I/O ----
    x_t = nc.dram_tensor("x_t", [NSB, 128, DT, SB], BF16,
                         kind="ExternalInput").ap()
    w_t = nc.dram_tensor("w_t", [NW, 128, DT, 128], BF16,
                         kind="ExternalInput").ap()
    wo_t = nc.dram_tensor("wo_t", [8, 128, HPG, 512], BF16,
                          kind="ExternalInput").ap()
    cosT = nc.dram_tensor("cosT", [NSB, 128, SB], F16, kind="ExternalInput").ap()
    sinT = nc.dram_tensor("sinT", [NSB, 128, SB], F16, kind="ExternalInput").ap()
    permT = nc.dram_tensor("permT", [128, 128], BF16, kind="ExternalInput").ap()
    maskT = nc.dram_tensor("maskT", [128, 128], BF16, kind="ExternalInput").ap()
    ones_col = nc.dram_tensor("ones_col", [128, 1], BF16, kind="ExternalInput").ap()
    ones_row = nc.dram_tensor("ones_row", [1, 128], F16, kind="ExternalInput").ap()
    ident = nc.dram_tensor("ident", [128, 128], BF16, kind="ExternalInput").ap()
    out = nc.dram_tensor("out", [NSB, 8, 4, 128, 512], F16,
                         kind="ExternalOutput").ap()

    with tile.TileContext(nc) as tc:
        with (
            tc.tile_pool(name="pers", bufs=1) as pers,
            tc.tile_pool(name="work", bufs=1) as wk,
            tc.tile_pool(name="psum", bufs=1, space="PSUM") as psum,
        ):
            # long-lived SBUF tensors
            kt_sb = pers.tile([128, KVPG, S], BF16, tag="kt")      # K^T roped
            v_sb = pers.tile([128, NKT, KVPG * 128], BF16, tag="v")  # V natural
            perm_sb = pers.tile([128, 128], BF16, tag="perm")
            mask_sb = pers.tile([128, 128], BF16, tag="mask")
            onec_sb = pers.tile([128, 1], BF16, tag="onec")
            oner_sb = pers.tile([1, 128], F16, tag="oner")
            id_sb = pers.tile([128, 128], BF16, tag="ident")
            wo_all = pers.tile([128, 8, HPG, 512], BF16, tag="wo")  # resident wo
            nc.gpsimd.dma_start(out=id_sb, in_=ident)
            nc.gpsimd.dma_start(out=perm_sb, in_=permT)
            nc.gpsimd.dma_start(out=mask_sb, in_=maskT)
            nc.gpsimd.dma_start(out=onec_sb, in_=ones_col)
            nc.gpsimd.dma_start(out=oner_sb, in_=ones_row)

            # dummy exp so the ~2.7us ACT table load happens during P(0)
            # (ACT is idle there) instead of stalling the first A task
            warm_i = pers.tile([1, 8], F32, tag="warmi")
            warm_o = pers.tile([1, 8], F32, tag="warmo")
            nc.vector.memset(warm_i, 0.0)
            nc.scalar.activation(warm_o, warm_i,
                                 mybir.ActivationFunctionType.Exp)

            def load_rope(sb):
                cb = wk.tile([128, SB], F16, tag="cosb", bufs=2, name="cosb")
                sb_ = wk.tile([128, SB], F16, tag="sinb", bufs=2, name="sinb")
                nc.gpsimd.dma_start(out=cb, in_=cosT[sb])
                nc.gpsimd.dma_start(out=sb_, in_=sinT[sb])
                return cb, sb_

            rope_tbl = load_rope(0)

            def load_xp(sb, chunks=(slice(0, 16), slice(16, 32))):
                xp = wk.tile([128, DT, SB], BF16, tag="xp", bufs=1, name="xp")
                for c in chunks:
                    nc.gpsimd.dma_start(out=xp[:, c, :], in_=x_t[sb, :, c, :])
                return xp

            def load_wh(w_idx, split=1):
                wh = wk.tile([128, DT, 128], BF16, tag="wh", bufs=2, name="wh")
                n = DT // split
                for hc in range(split):
                    nc.sync.dma_start(
                        out=wh[:, hc * n:(hc + 1) * n, :],
                        in_=w_t[w_idx, :, hc * n:(hc + 1) * n, :])
                return wh

            # startup: sync is dedicated to the x panel (the first proj
            # chains sweep all of it), wh0 on scalar, wh1 on gpsimd; the
            # first two wo blocks queue behind everything critical
            wh0 = wk.tile([128, DT, 128], BF16, tag="wh", bufs=2, name="wh")
            wh1 = wk.tile([128, DT, 128], BF16, tag="wh", bufs=2, name="wh")
            xp = wk.tile([128, DT, SB], BF16, tag="xp", bufs=1, name="xp")
            for dblk in range(2):
                nc.gpsimd.dma_start(out=wo_all[:, dblk], in_=wo_t[dblk])
            for c in range(8):
                cs = slice(c * 4, (c + 1) * 4)
                nc.sync.dma_start(out=xp[:, cs, :], in_=x_t[0, :, cs, :])
                nc.scalar.dma_start(out=wh0[:, cs, :], in_=w_t[0, :, cs, :])
                nc.scalar.dma_start(out=wh1[:, cs, :], in_=w_t[1, :, cs, :])
            pending_wh = {0: wh0, 1: wh1}
            o_work = []
            for sb in range(NSB):
                scols = slice(sb * SB, (sb + 1) * SB)
                cos_blk, sin_blk = rope_tbl
                if sb + 1 < NSB:
                    rope_tbl = load_rope(sb + 1)

                # ============ P(sb): QKV projection + RoPE ============
                rope_pending = []

                def flush_rope_one():
                    raw, dst = rope_pending.pop(0)
                    pp = psum.tile([128, SB], F32, tag="mm", bufs=2,
                                   padded_shape=[128, SB * 2], name="pp")
                    nc.tensor.matmul(pp, perm_sb, raw, start=True, stop=True)
                    nc.vector.tensor_mul(dst, pp, sin_blk)
                    nc.vector.tensor_add(dst, dst, t1s.pop(id(raw)))

                def flush_rope():
                    while rope_pending:
                        flush_rope_one()

                t1s = {}

                def rope_head(acc, w_idx):
                    # PSUM -> raw/t1, queue the rope flush for this head
                    raw = wk.tile([128, SB], BF16, tag="raw", bufs=2)
                    nc.vector.tensor_copy(raw, acc)
                    t1 = wk.tile([128, SB], F32, tag="t1", bufs=2)
                    nc.vector.tensor_mul(t1, raw, cos_blk)
                    t1s[id(raw)] = t1
                    if w_idx < 2:
                        dst = kt_sb[:, w_idx, scols]
                    else:
                        dst = qt[:, w_idx - 4, :]
                    rope_pending.append((raw, dst))
                    if len(rope_pending) > 1:
                        flush_rope_one()

                if sb == 0:
                    # the first chain is DMA-throttled (one head sweeps the
                    # x panel at ~2x fabric rate); interleaving the two
                    # k-heads halves the per-chunk demand rate
                    whA = pending_wh.pop(0)
                    whB = pending_wh.pop(1)
                    accA = psum.tile([128, SB], F32, tag="acc", bufs=2,
                                     name="acc")
                    accB = psum.tile([128, SB], F32, tag="acc", bufs=2,
                                     name="acc")
                    for dt_i in range(DT):
                        nc.tensor.matmul(
                            accA, whA[:, dt_i, :], xp[:, dt_i, :],
                            start=(dt_i == 0), stop=(dt_i == DT - 1))
                        nc.tensor.matmul(
                            accB, whB[:, dt_i, :], xp[:, dt_i, :],
                            start=(dt_i == 0), stop=(dt_i == DT - 1))
                    rope_head(accA, 0)
                    rope_head(accB, 1)
                    head_list = range(2, NW)
                else:
                    head_list = range(NW)

                for w_idx in head_list:
                    wh = pending_wh.pop(w_idx, None)
                    if wh is None:
                        wh = load_wh(w_idx)
                    if 2 <= w_idx < 4:
                        # v head: [hd, s] proj then PE-transpose to natural
                        kvs = w_idx - 2
                        acc = psum.tile([128, SB], F32, tag="acc", bufs=2,
                                        name="acc")
                        for dt_i in range(DT):
                            nc.tensor.matmul(
                                acc, wh[:, dt_i, :], xp[:, dt_i, :],
                                start=(dt_i == 0), stop=(dt_i == DT - 1))
                        vtp = wk.tile([128, SB], BF16, tag="vtp", bufs=1)
                        nc.vector.tensor_copy(vtp, acc)
                        for blk in range(SB // 128):
                            tp = psum.tile([128, 128], BF16, tag="mm", bufs=2,
                                           padded_shape=[128, SB * 4], name="tp")
                            nc.tensor.transpose(
                                tp, vtp[:, blk * 128:(blk + 1) * 128], id_sb)
                            nc.vector.tensor_copy(
                                v_sb[:, sb * 4 + blk,
                                     kvs * 128:(kvs + 1) * 128], tp)
                    else:
                        # q or k head: [hd, s] layout + rope
                        acc = psum.tile([128, SB], F32, tag="acc", bufs=2)
                        for dt_i in range(DT):
                            nc.tensor.matmul(
                                acc, wh[:, dt_i, :], xp[:, dt_i, :],
                                start=(dt_i == 0), stop=(dt_i == DT - 1))
                        if w_idx == 4:
                            qt = wk.tile([128, HPG, SB], BF16, tag="qt",
                                         bufs=1)
                        rope_head(acc, w_idx)

                flush_rope()

                if sb == 0:
                    # remaining wo blocks, queued after P(0)'s weight stream
                    for dblk in range(2, 8):
                        nc.sync.dma_start(out=wo_all[:, dblk], in_=wo_t[dblk])

                # prefetch next s-block activations; the DMA starts as soon
                # as P(sb)'s last read of the single xp buffer retires and
                # hides under A(sb)+O(sb)
                if sb + 1 < NSB:
                    xp = load_xp(sb + 1)

                # ============ A(sb): attention q-block qi=sb ============
                # Scores/exp for head h+1 are interleaved (cross-head
                # software pipeline) with PV/l-row of head h so the tensor
                # engine never idles waiting on the ACT exp stream (idle
                # gaps re-engage the HAM clock throttle).
                nkt = 4 * sb + 4

                def q0(kti):  # first causally-live query column for key tile
                    return 128 * (kti - 4 * sb) if kti >= 4 * sb else 0

                # every task is a pair of key tiles sharing one wide exp;
                # diagonal pairs exp from the wider tile's first live column
                # (the partner's dead columns are never read downstream)
                tasks = []
                for h in range(HPG):
                    tasks += [(h, (k, k + 1)) for k in range(0, nkt, 2)]
                pt_tiles = {}
                state = {"ti": 0, "tiles": 0}

                def issue_task():
                    if state["ti"] >= len(tasks):
                        return
                    h2, ks = tasks[state["ti"]]
                    state["ti"] += 1
                    state["tiles"] += 2
                    if ks[0] == 0:
                        pt_tiles[h2] = wk.tile([128, NKT, SB], BF16, tag="pt",
                                               bufs=2, name="pt")
                    pt = pt_tiles[h2]
                    kvs2 = h2 // (HPG // KVPG)
                    st = psum.tile([128, 2, SB], F32, tag="mm", bufs=2,
                                   name="st")
                    for j, k in enumerate(ks):
                        qo = q0(k)
                        nc.tensor.matmul(
                            st[:, j, qo:],
                            kt_sb[:, kvs2, k * 128:(k + 1) * 128],
                            qt[:, h2, qo:],
                            start=True, stop=True)
                    eqo = q0(ks[0])
                    nc.scalar.activation(
                        pt[:, ks[0]:ks[0] + 2, eqo:], st[:, :, eqo:],
                        mybir.ActivationFunctionType.Exp, scale=SCALE)
                    for k in ks:
                        if k >= 4 * sb:  # diagonal tile: mask its 128-chunk
                            qo = q0(k)
                            nc.gpsimd.tensor_mul(
                                pt[:, k, qo:qo + 128],
                                pt[:, k, qo:qo + 128], mask_sb)

                def emit_o_tile(at_prev, psb, dblk, qs):
                    # one o-proj output tile of s-block psb (PE filler work)
                    ops = psum.tile([128, 512], F32, tag="alt", bufs=2,
                                    name="ops")
                    for hp in range(HPG):
                        nc.tensor.matmul(
                            ops,
                            at_prev[:, hp, qs * 128:(qs + 1) * 128],
                            wo_all[:, dblk, hp, :],
                            start=(hp == 0), stop=(hp == HPG - 1))
                    o_sb = wk.tile([128, 512], F16, tag="osb", bufs=4)
                    nc.vector.tensor_copy(o_sb, ops)
                    nc.scalar.dma_start(out=out[psb, dblk, qs], in_=o_sb)

                # two filler tiles up front cover the exp pipeline-fill and
                # rope-tail latency at the start of A(sb)
                for _ in range(2):
                    if o_work:
                        emit_o_tile(*o_work.pop(0))
                while state["tiles"] < 4 and state["ti"] < len(tasks):
                    issue_task()
                attnT = wk.tile([128, HPG, SB], BF16, tag="attnT", bufs=2)
                consumed = 0
                for h in range(HPG):
                    kvs = h // (HPG // KVPG)
                    pt = pt_tiles[h]
                    oT = psum.tile([128, SB], F32, tag="acc", bufs=2)
                    lrow = psum.tile([1, SB], F32, tag="alt", bufs=2,
                                     name="lrow")
                    # softmax denominator: partial-sum all key tiles on the
                    # DVE in bf16 (2x rate), fold across partitions with one
                    # ones-matmul per head
                    lp = wk.tile([128, SB], BF16, tag="lp", bufs=1)
                    for kti in range(nkt):
                        qo = q0(kti)
                        nc.tensor.matmul(
                            oT[:, qo:],
                            v_sb[:, kti, kvs * 128:(kvs + 1) * 128],
                            pt[:, kti, qo:],
                            start=(kti == 0), stop=(kti == nkt - 1))
                        if kti == 0:
                            nc.vector.tensor_copy(lp, pt[:, 0, :])
                        else:
                            nc.vector.tensor_add(
                                lp[:, qo:], lp[:, qo:], pt[:, kti, qo:])
                        consumed += 1
                        while (state["tiles"] < consumed + 4
                               and state["ti"] < len(tasks)):
                            issue_task()
                    nc.tensor.matmul(lrow, onec_sb, lp,
                                     start=True, stop=True)
                    # finalize: attnT = oT * bcast(1/l).  One o-proj filler
                    # tile of the previous s-block hides the DVE reciprocal
                    # latency between the fold and bc matmuls; three more
                    # keep the PE fed while the ACT exp stream catches up.
                    # (recip is issued BEFORE the filler so the filler's DVE
                    # drain doesn't delay it on the in-order DVE queue)
                    linv = wk.tile([1, SB], F32, tag="linv", bufs=1)
                    nc.vector.reciprocal_approx_fast(linv, lrow)
                    linv_r = wk.tile([1, SB], F16, tag="linvr", bufs=1)
                    nc.vector.tensor_copy(linv_r, linv)
                    if o_work:
                        emit_o_tile(*o_work.pop(0))
                    bc = psum.tile([128, SB], F32, tag="alt", bufs=2,
                                   name="bc")
                    nc.tensor.matmul(bc, oner_sb, linv_r,
                                     start=True, stop=True)
                    bc_sb = wk.tile([128, SB], F16, tag="bcsb", bufs=1)
                    nc.vector.tensor_copy(bc_sb, bc)
                    nc.vector.tensor_mul(attnT[:, h, :], oT, bc_sb)
                    for _ in range(3):
                        if o_work:
                            emit_o_tile(*o_work.pop(0))

                # o-proj of THIS s-block: deferred into A(sb+1) as PE filler
                # (the ACT-bound exp stream there leaves PE slack); the last
                # block's runs right here
                while o_work:
                    emit_o_tile(*o_work.pop(0))
                o_work = [(attnT, sb, dblk, qs)
                          for dblk in range(8) for qs in range(4)]
                if sb + 1 == NSB:
                    while o_work:
                        emit_o_tile(*o_work.pop(0))

                # prefetch next s-block weights ahead of the out-DMA backlog
                if sb + 1 < NSB:
                    pending_wh = {0: load_wh(0), 1: load_wh(1)}
    nc.compile()
    return nc


def _host_inputs(x, wq, wk, wv, wo, cos, sin):
    """Build the 8 per-core input maps (all host-side prep)."""
    x = np.asarray(x, np.float32)
    wq = np.asarray(wq, np.float32)
    wk = np.asarray(wk, np.float32)
    wv = np.asarray(wv, np.float32)
    wo = np.asarray(wo, np.float32)
    cos = np.asarray(cos, np.float32)
    sin = np.asarray(sin, np.float32)

    # [NSB, 128, SB] fp16 per-block rotary tables (rows repeated in pairs)
    cosT = np.ascontiguousarray(
        np.repeat(cos.T, 2, axis=0).reshape(128, NSB, SB).transpose(1, 0, 2)
    ).astype(np.float16)
    sinT = np.ascontiguousarray(
        np.repeat(sin.T, 2, axis=0).reshape(128, NSB, SB).transpose(1, 0, 2)
    ).astype(np.float16)
    permT = np.zeros((128, 128), ml_dtypes.bfloat16)
    idx = np.arange(64)
    permT[2 * idx + 1, 2 * idx] = -1.0
    permT[2 * idx, 2 * idx + 1] = 1.0
    kk = np.arange(128)[:, None]
    tt = np.arange(128)[None, :]
    maskT = (tt >= kk).astype(ml_dtypes.bfloat16)      # [128, 128]
    ones_col = np.ones((128, 1), ml_dtypes.bfloat16)
    ones_row = np.ones((1, 128), np.float16)
    ident = np.eye(128, dtype=ml_dtypes.bfloat16)

    def tile_w(w_col):  # [D, 128] -> [128, DT, 128]
        return w_col.reshape(DT, 128, 128).transpose(1, 0, 2)

    x_ts = [np.ascontiguousarray(
        x[b].T.reshape(DT, 128, NSB, SB).transpose(2, 1, 0, 3)
    ).astype(ml_dtypes.bfloat16) for b in range(B)]
    in_maps = []
    for core in range(N_CORES):
        b, g = divmod(core, G)
        w_np = np.empty((NW, 128, DT, 128), np.float32)
        for j in range(KVPG):
            w_np[j] = tile_w(wk[:, (g * KVPG + j) * 128:(g * KVPG + j + 1) * 128])
        for j in range(KVPG):
            w_np[KVPG + j] = tile_w(
                wv[:, (g * KVPG + j) * 128:(g * KVPG + j + 1) * 128])
        for j in range(HPG):
            w_np[2 * KVPG + j] = tile_w(
                wq[:, (g * HPG + j) * 128:(g * HPG + j + 1) * 128])
        wo_g = wo[g * HPG * HD:(g + 1) * HPG * HD, :]          # [1024, D]
        wo_np = np.ascontiguousarray(
            wo_g.reshape(HPG, 128, 8, 512).transpose(2, 1, 0, 3)
        ).astype(ml_dtypes.bfloat16)                           # [8, 128, HPG, 512]
        in_maps.append({
            "x_t": x_ts[b], "w_t": w_np.astype(ml_dtypes.bfloat16),
            "wo_t": wo_np,
            "cosT": cosT, "sinT": sinT, "permT": permT, "maskT": maskT,
            "ones_col": ones_col, "ones_row": ones_row, "ident": ident,
        })
    return in_maps


def kernel(x, wq, wk, wv, wo, cos, sin, mask, start_pos):
    assert int(start_pos) == 0, "kernel compiled for prefill (start_pos=0)"
    if "nc" not in _CACHE:
        _CACHE["nc"] = _build()
    nc = _CACHE["nc"]
    in_maps = _host_inputs(x, wq, wk, wv, wo, cos, sin)
    res = run_bass_kernel_spmd(nc, in_maps, list(range(N_CORES)))

    def unpack(o):  # [NSB, 8, 4, 128, 512] -> [S, D]
        return np.ascontiguousarray(
            np.transpose(o, (0, 2, 3, 1, 4)).reshape(S, D))

    full = np.empty((B, S, D), np.float32)
    for b in range(B):
        acc = res.results[4 * b]["out"].astype(np.float32)
        for g in range(1, G):
            acc = acc + res.results[4 * b + g]["out"]
        full[b] = unpack(acc)
    return full

